# revision 38
# baseline (speedup 1.0000x reference)
"""Trainium2 Bass kernel for a dense transformer block (B=4, N=2048, C=768,
H=12, D=64, HID=3072), sharded over 8 NeuronCores.

Sharding: token-split, no collectives. Core s handles batch b = s//2,
sequence half = s%2 (1024 tokens). Each core receives its batch element's
full 2048-token x (rolled so its own tokens are rows 0..1023), computes
K/V over all 2048 tokens (redundantly with its pair core), and produces
the output for its own 1024 tokens. Host gathers/transposes.

v3 schedule (trace-driven rewrite of v2):
- era1 interleaves per 512-token group: x DMA -> raw-x^T transposes (PE
  work with no LN dependency) -> LN1 -> h^T transposes -> V -> K/Q
  chunks, so PE/Scalar/DVE overlap from t~2us. Consts land in ONE
  batched DMA (cvec).
- attention is ScalarE(exp)-bound; PE fillers are fine-grained QUANTA
  (1 matmul each) pulled one-per-step inside the S->exp->AV pipeline,
  with per-head-pair deadlines (kq ft3-5 feed chunk0's own later head
  pairs; proj/LN2/h2/fc1-staging of chunk0 run during chunk1).
- softmax denominators: batched reciprocal split in two halves per
  chunk so oT normalization starts 3 head-pairs early and proj quanta
  are ready at chunk1 start / tail start.
- all rstd use Exp(-0.5*Ln(var+eps)) -- same activation table set as
  the softmax exp (natural_log_exp_and_others); Gelu is the only other
  set, so 2 ACT table loads total instead of 12.
- weights prefetch: wproj+wfc1(h0) at chunk0 start, wfc1(h1) at chunk1
  start, wfc2(h1) at tail start -- tail never waits on DMA.
- tail order: proj(1) -> ln2(1)mm -> fc2(0,0) -> fc1(0,1) -> fc2(0,1)
  -> fc1(1,0) -> fc2(1,0) -> fc1(1,1) -> fc2(1,1), gelu fused from
  PSUM, so the LN2(1) serial chain hides under fc2/fc1 matmuls.
"""

from contextlib import ExitStack

import numpy as np

import concourse.bass as bass
import concourse.mybir as mybir
import concourse.tile as tile
from concourse import bacc
from concourse.bass_utils import run_bass_kernel_spmd
from concourse.masks import make_identity

F32 = mybir.dt.float32
BF16 = mybir.dt.bfloat16
AF = mybir.ActivationFunctionType
ALU = mybir.AluOpType

B, N, C = 4, 2048, 768
H, D = 12, 64
HID = 3072
EPS = 1e-5
NCORES = 8
NO = 1024  # tokens owned per core
NKV = 2048  # key/value tokens per core
CT = C // 128  # 6 feature tiles
HT = HID // 128  # 24 hidden tiles
HQ = HT // 2  # 12 hidden tiles per half
KT = NKV // 128  # 16 kv token tiles
QCH = NO // 512  # 2 query chunks of 512
ISCALE = 1.0 / np.sqrt(D)
SKEW = 2  # S->AV software-pipeline depth (in nt steps)
NCV = 36 + HT  # cvec columns: g1,b1,g2,b2,pb,f2b (6 ea) + f1b (24)

LAST_RESULTS = None
_NC_CACHE = None


def build_program(repeats=1):
    nc = bacc.Bacc(trn_type="TRN2", target_bir_lowering=False, num_devices=NCORES)

    xbT = nc.dram_tensor("xbT", [C, NKV], BF16, kind="ExternalInput").ap()
    wqkvT = nc.dram_tensor("wqkvT", [C, 3 * C], BF16, kind="ExternalInput").ap()
    wprojT = nc.dram_tensor("wprojT", [C, C], BF16, kind="ExternalInput").ap()
    wfc1T = nc.dram_tensor("wfc1T", [C, HID], BF16, kind="ExternalInput").ap()
    wfc2T = nc.dram_tensor("wfc2T", [HID, C], BF16, kind="ExternalInput").ap()
    cvec = nc.dram_tensor("cvec", [128, NCV], F32, kind="ExternalInput").ap()
    outT = nc.dram_tensor("outT", [C, NO], F32, kind="ExternalOutput").ap()

    with tile.TileContext(nc) as tc:
        for _ in range(repeats):
            emit(nc, tc, xbT, wqkvT, wprojT, wfc1T, wfc2T, cvec, outT)
    nc.compile()
    return nc


def emit(nc, tc, xbT, wqkvT, wprojT, wfc1T, wfc2T, cvec, outT):
    dma = nc.sync.dma_start
    qs = (slice(0, 64), slice(64, 128))

    with ExitStack() as _st:
        def pool(**kw):
            return _st.enter_context(tc.tile_pool(**kw))

        consts = pool(name="consts", bufs=1)
        x2T_pool = pool(name="x2T", bufs=1)
        xoT_pool = pool(name="xoT", bufs=1)

        # ---- constants: one batched DMA for every small vector ----
        cv = consts.tile([128, NCV], F32, tag="cv")
        dma(out=cv, in_=cvec)
        g1_s = cv[:, 0:6]
        b1_s = cv[:, 6:12]
        g2_s = cv[:, 12:18]
        b2_s = cv[:, 18:24]
        pb_s = cv[:, 24:30]
        f2b_s = cv[:, 30:36]
        f1b_s = cv[:, 36:36 + HT]

        ones_f32 = consts.tile([128, 1], F32, tag="ones_f32")
        nc.vector.memset(ones_f32, 1.0)
        ones_cb = consts.tile([128, 1], BF16, tag="ones_cb")
        nc.scalar.activation(out=ones_cb, in_=ones_f32, func=AF.Copy)
        eps_t = consts.tile([128, 1], F32, tag="eps")
        nc.vector.memset(eps_t, EPS)

        # residual streams (bf16 to fit SBUF; rel-err budget is 2e-2).
        # xoT holds the host-transposed raw x^T for the OWN tokens -- it
        # doubles as the residual stream (no PE transposes anywhere).
        x2T = [x2T_pool.tile([128, NO], BF16, tag=f"x2T{ct}", name=f"x2T{ct}")
               for ct in range(CT)]
        xoT = [xoT_pool.tile([128, NO], BF16, tag=f"xoT{ct}", name=f"xoT{ct}")
               for ct in range(CT)]

        # ---- PSUM pools: sps 4 banks, ops 2, mps 2 (8 total) ----
        sps = pool(name="s_psum", bufs=2, space="PSUM")
        ops = pool(name="o_psum", bufs=2, space="PSUM")
        mps = pool(name="m_psum", bufs=2, space="PSUM")

        oU_pool = pool(name="oU", bufs=12)
        oT_pool = pool(name="oT", bufs=6)
        den_pool = pool(name="den", bufs=1)
        asm = pool(name="asm", bufs=2)

        _stk = ExitStack()  # attention working set: freed at tail

        def kpool(**kw):
            return _stk.enter_context(tc.tile_pool(**kw))

        qT_pool = kpool(name="qT", bufs=1)
        kT_pool = kpool(name="kT", bufs=1)
        vA_pool = kpool(name="vA", bufs=1)
        p_sb = kpool(name="p_sb", bufs=4)
        qT = [qT_pool.tile([128, NO], BF16, tag=f"qT{ct}", name=f"qT{ct}")
              for ct in range(CT)]
        kT = [kT_pool.tile([128, NKV], BF16, tag=f"kT{ct}", name=f"kT{ct}")
              for ct in range(CT)]
        vA = [vA_pool.tile([128, H, D + 1], BF16, tag=f"vA{nt}",
                           name=f"vA{nt}") for nt in range(KT)]

        dent = [None, None]
        oT = [[None] * CT for _ in range(QCH)]
        oU_all = [[None] * H for _ in range(QCH)]
        ln2v = [None, None]
        h2c = [[None] * CT for _ in range(QCH)]

        _hstk = ExitStack()  # hkvT/wq/xkv: freed at chunk0|chunk1 boundary
        hkvT_pool = _hstk.enter_context(tc.tile_pool(name="hkvT", bufs=1))
        wq_pool = _hstk.enter_context(tc.tile_pool(name="wqkv", bufs=1))
        xkv_pool = _hstk.enter_context(tc.tile_pool(name="xkv", bufs=1))
        hkvT = [hkvT_pool.tile([128, NKV], BF16, tag=f"hkvT{ct}",
                               name=f"hkvT{ct}") for ct in range(CT)]
        # x^T for the kv-only tokens (own-token half lives in xoT)
        xkv = [xkv_pool.tile([128, NO], BF16, tag=f"xkv{ct}",
                             name=f"xkv{ct}") for ct in range(CT)]

        def xT_chunk(ct, chk):
            """SBUF view of host-transposed x^T [128, 512] for chunk chk."""
            if chk < QCH:
                return xoT[ct][:, chk * 512:(chk + 1) * 512]
            return xkv[ct][:, (chk - QCH) * 512:(chk - QCH + 1) * 512]

        wq = wq_pool.tile([128, CT, 3 * C], BF16, tag="wqkv")
        # x^T tiles first on the sync queue (first consumers); wq rides the
        # scalar-engine DMA path in K,V,Q column order so x prefetch is
        # never stuck behind it and each slice lands just before its first
        # consumer in group 0.
        for ct in range(CT):
            dma(out=xoT[ct], in_=xbT[ct * 128:(ct + 1) * 128, 0:NO])
        for ct in range(CT):
            dma(out=xkv[ct], in_=xbT[ct * 128:(ct + 1) * 128, NO:NKV])
        _wq_src = wqkvT.rearrange("(ct p) f -> p ct f", p=128)
        nc.scalar.dma_start(out=wq[:, :, C:2 * C], in_=_wq_src[:, :, C:2 * C])
        nc.scalar.dma_start(out=wq[:, :, 2 * C:3 * C],
                            in_=_wq_src[:, :, 2 * C:3 * C])
        nc.scalar.dma_start(out=wq[:, :, 0:C], in_=_wq_src[:, :, 0:C])

        # =================== era1: per-group interleave ===================
        def v_tile(nt):
            """V for one kv token tile -> vA[nt] (sps psum, 3 banks used)."""
            psAB = sps.tile([128, 1024], F32, tag="ps", name=f"psAB{nt}")
            for ct in range(CT):
                hk = hkvT[ct][:, nt * 128:(nt + 1) * 128]
                nc.tensor.matmul(psAB[:, 0:512], hk, wq[:, ct, 2 * C:2 * C + 512],
                                 start=(ct == 0), stop=(ct == CT - 1))
                nc.tensor.matmul(psAB[:, 512:768], hk,
                                 wq[:, ct, 2 * C + 512:3 * C],
                                 start=(ct == 0), stop=(ct == CT - 1))
            nc.vector.tensor_copy(
                out=vA[nt][:, 0:8, 0:D],
                in_=psAB[:, 0:512].rearrange("p (h d) -> p h d", d=D))
            nc.vector.tensor_copy(
                out=vA[nt][:, 8:12, 0:D],
                in_=psAB[:, 512:768].rearrange("p (h d) -> p h d", d=D))
            nc.vector.memset(vA[nt][:, :, D:D + 1], 1.0)

        def k_chunk(ft, chk):
            ps = mps.tile([128, 512], F32, tag="mp", name=f"kc{ft}_{chk}")
            for ct in range(CT):
                nc.tensor.matmul(
                    ps, wq[:, ct, C + ft * 128:C + (ft + 1) * 128],
                    hkvT[ct][:, chk * 512:(chk + 1) * 512],
                    start=(ct == 0), stop=(ct == CT - 1))
                if ct < CT - 1:
                    yield
            nc.vector.tensor_copy(
                out=kT[ft][:, chk * 512:(chk + 1) * 512], in_=ps)
            yield

        def q_chunk(ft, chk):
            ps = mps.tile([128, 512], F32, tag="mp", name=f"qc{ft}_{chk}")
            for ct in range(CT):
                nc.tensor.matmul(
                    ps, wq[:, ct, ft * 128:(ft + 1) * 128],
                    hkvT[ct][:, chk * 512:(chk + 1) * 512],
                    start=(ct == 0), stop=(ct == CT - 1))
                if ct < CT - 1:
                    yield
            nc.vector.tensor_copy(
                out=qT[ft][:, chk * 512:(chk + 1) * 512], in_=ps)
            yield

        def run_gen(g):
            for _ in g:
                pass

        with (
            tc.tile_pool(name="ln1_work", bufs=1) as lw1,
            tc.tile_pool(name="ln1_bc", bufs=2) as bc1,
        ):
            for g in range(NKV // 512):  # 512-token groups
                cs = slice(g * 512, (g + 1) * 512)
                # LN1 stats over features (partition dim) via ones-matmuls
                psum = mps.tile([1, 512], F32, tag="mp", name="l1sum")
                pssq = mps.tile([1, 512], F32, tag="mp", name="l1ssq")
                for ct in range(CT):
                    xg = xT_chunk(ct, g)
                    sq = lw1.tile([128, 512], BF16, tag="sq")
                    nc.vector.tensor_mul(sq, xg, xg)
                    nc.tensor.matmul(psum, ones_cb, xg,
                                     start=(ct == 0), stop=(ct == CT - 1),
                                     skip_group_check=True)
                    nc.tensor.matmul(pssq, ones_cb, sq,
                                     start=(ct == 0), stop=(ct == CT - 1),
                                     skip_group_check=True)
                mu = lw1.tile([1, 512], F32, tag="mu")
                nc.vector.tensor_scalar_mul(mu, psum, 1.0 / C)
                mu2 = lw1.tile([1, 512], F32, tag="mu2")
                nc.vector.tensor_mul(mu2, mu, mu)
                var = lw1.tile([1, 512], F32, tag="var")
                nc.vector.scalar_tensor_tensor(
                    out=var, in0=pssq, scalar=1.0 / C, in1=mu2,
                    op0=ALU.mult, op1=ALU.subtract)
                std = lw1.tile([1, 512], F32, tag="mu2", name="std1")
                nc.scalar.activation(out=std, in_=var, func=AF.Sqrt,
                                     bias=eps_t[0:1], scale=1.0)
                rstdf = lw1.tile([1, 512], F32, tag="rstdf")
                nc.vector.reciprocal(out=rstdf, in_=std)
                rstd = lw1.tile([1, 512], BF16, tag="rstd1")
                nc.vector.tensor_copy(out=rstd, in_=rstdf)
                nmr = lw1.tile([1, 512], BF16, tag="nmr1")
                nc.vector.scalar_tensor_tensor(
                    out=nmr, in0=mu, scalar=-1.0, in1=rstdf,
                    op0=ALU.mult, op1=ALU.mult)
                bc_r = bc1.tile([128, 512], BF16, tag="bc_r")
                nc.gpsimd.partition_broadcast(bc_r, rstd, channels=128)
                bc_m = bc1.tile([128, 512], BF16, tag="bc_m")
                nc.gpsimd.partition_broadcast(bc_m, nmr, channels=128)
                # h^T = (x^T*rstd + nmr)*g1 + b1: token-affine on DVE,
                # feature-affine on ScalarE
                for ct in range(CT):
                    t = lw1.tile([128, 512], BF16, tag="h1tmp")
                    nc.vector.tensor_mul(t, xT_chunk(ct, g), bc_r)
                    nc.vector.tensor_add(t, t, bc_m)
                    nc.scalar.activation(
                        out=hkvT[ct][:, cs], in_=t, func=AF.Identity,
                        scale=g1_s[:, ct:ct + 1], bias=b1_s[:, ct:ct + 1])
                for ft in range(3):  # ft0-2 K here; ft3-5 are attn fillers
                    run_gen(k_chunk(ft, g))
                for j in range(4):
                    v_tile(4 * g + j)
                if g < QCH:
                    for ft in range(3):
                        run_gen(q_chunk(ft, g))

        # =================== attention machinery ===================
        def ot_norm_piece(ch, hp, rec, base):
            """Normalize one head pair's output into oT[ch][hp]."""
            t = oT_pool.tile([128, 512], BF16, tag="oT",
                             name=f"oT{ch}_{hp}")
            for i in range(2):
                r = 2 * hp + i
                rb = asm.tile([1, 512], BF16, tag="rb")
                # gpsimd-queue DMA: gpsimd is idle, and this never
                # head-of-line blocks the sync queue's big transfers
                nc.gpsimd.dma_start(out=rb, in_=rec[r - base:r - base + 1, :])
                vb = asm.tile([D, 512], BF16, tag="vb")
                nc.gpsimd.partition_broadcast(vb, rb, channels=D)
                nc.vector.tensor_mul(t[qs[i], :], oU_all[ch][r], vb)
            oT[ch][hp] = t

        def rec_quarter(ch, q):
            """Batched reciprocal of 4 denominator rows (heads 4q..4q+3),
            then normalize head pairs 2q and 2q+1 -- runs as soon as each
            third of a chunk's denominators lands so oT is ready early."""
            rec = dent[ch][q]
            with nc.allow_low_precision(reason="softmax denom in bf16"):
                nc.vector.reciprocal(out=rec, in_=rec)
            for hp in (2 * q, 2 * q + 1):
                ot_norm_piece(ch, hp, rec, 4 * q)

        class Feed:
            """Deadline-ordered filler quanta. pull() emits one quantum;
            flush(hp) force-emits every generator due before head pair hp
            (PE is in-order, so a quantum consumed by hp's matmuls must be
            emitted before them)."""

            def __init__(self):
                self.items = []

            def add(self, deadline, gen):
                self.items.append([deadline, gen])

            def pull(self):
                for it in self.items:
                    if it[1] is not None:
                        try:
                            next(it[1])
                            return True
                        except StopIteration:
                            it[1] = None
                return False

            def flush(self, hp):
                for it in self.items:
                    if it[1] is not None and it[0] <= hp:
                        for _ in it[1]:
                            pass
                        it[1] = None

            def flush_all(self):
                self.flush(10 ** 9)

        def attention_all(feed_of, boundary_hook):
            """Both chunks as ONE continuously-skewed (ch, hp, nt) stream:
            the next head pair's S matmuls interleave with the previous
            pair's AV tail, so the exp stream never pauses at boundaries.
            feed_of(ch) -> Feed; boundary_hook(ch) fires before chunk ch's
            first S (pool swaps / weight prefetch / feed construction)."""
            seq = [(ch, hp, nt) for ch in range(QCH)
                   for hp in range(CT) for nt in range(KT)]
            po_cur = {}
            pts = {}
            feed = None
            for idx in range(len(seq) + SKEW):
                if idx < len(seq):
                    ch, hp, nt = seq[idx]
                    if nt == 0:
                        if hp == 0:
                            if feed is not None:
                                feed.flush_all()
                            boundary_hook(ch)
                            feed = feed_of(ch)
                            dent[ch] = [
                                den_pool.tile([4, 512], BF16, tag=f"dent{h}",
                                              name=f"dent{ch}_{h}")
                                for h in range(3)]
                        feed.flush(hp)
                    qch = slice(ch * 512, (ch + 1) * 512)
                    ps2 = sps.tile([128, 1024], F32, tag="ps")
                    for i in range(2):
                        nc.tensor.matmul(
                            ps2[:, i * 512:(i + 1) * 512],
                            kT[hp][qs[i], nt * 128:(nt + 1) * 128],
                            qT[hp][qs[i], qch],
                            start=True, stop=True,
                            tile_position=(64 * i, 0))
                    pt2 = p_sb.tile([128, 1024], BF16, tag="pt")
                    nc.scalar.activation(out=pt2, in_=ps2,
                                         func=AF.Exp, scale=ISCALE)
                    pts[(ch, hp, nt)] = pt2
                if idx >= SKEW:
                    ch_a, hp_a, m = seq[idx - SKEW]
                    pt2 = pts.pop((ch_a, hp_a, m))
                    if m == 0:
                        po_cur[(ch_a, hp_a)] = [
                            ops.tile([D + 1, 512], F32, tag="po",
                                     name=f"po{ch_a}_{hp_a}_{i}")
                            for i in range(2)]
                    po = po_cur[(ch_a, hp_a)]
                    for i in range(2):
                        nc.tensor.matmul(
                            po[i], vA[m][:, 2 * hp_a + i, :],
                            pt2[:, i * 512:(i + 1) * 512],
                            start=(m == 0), stop=(m == KT - 1),
                            skip_group_check=True)
                    if m == KT - 1:
                        # drain: unnormalized O^T halves + denominator row
                        for i in range(2):
                            r = 2 * hp_a + i
                            oU = oU_pool.tile([D, 512], BF16, tag="oU",
                                              name=f"oU{ch_a}_{r}")
                            nc.vector.tensor_copy(out=oU, in_=po[i][0:D, :])
                            oU_all[ch_a][r] = oU
                            dt = asm.tile([1, 512], BF16, tag="dtmp")
                            nc.vector.tensor_copy(out=dt, in_=po[i][D:D + 1, :])
                            nc.gpsimd.dma_start(
                                out=dent[ch_a][r // 4][(r % 4):(r % 4) + 1, :],
                                in_=dt)
                        del po_cur[(ch_a, hp_a)]
                        if hp_a % 2 == 1:
                            rec_quarter(ch_a, hp_a // 2)
                # filler pulls: ~1.5/step keeps PE full without starving
                # the exp stream, and drains inventory before chunk end
                feed.pull()
                if idx % 2 == 0:
                    feed.pull()

        # ------------- proj / LN2 / MLP building blocks -------------
        def proj_quanta(ch, wp):
            """proj + x2 residual for chunk ch. f0/f1 start on the first
            five oT tiles so quanta are ready before oT[5] lands."""
            cs = slice(ch * 512, (ch + 1) * 512)
            ps01 = []
            for f in range(2):
                ps = mps.tile([128, 512], F32, tag="mp", name=f"pj{f}")
                ps01.append(ps)
                for ct in range(CT - 1):
                    nc.tensor.matmul(
                        ps, wp[:, ct, f * 128:(f + 1) * 128], oT[ch][ct],
                        start=(ct == 0), stop=False)
                    yield
            for f in range(2):
                nc.tensor.matmul(
                    ps01[f], wp[:, CT - 1, f * 128:(f + 1) * 128],
                    oT[ch][CT - 1], start=False, stop=True)
                nc.vector.scalar_tensor_tensor(
                    out=x2T[f][:, cs], in0=ps01[f],
                    scalar=pb_s[:, f:f + 1], in1=xoT[f][:, cs],
                    op0=ALU.add, op1=ALU.add)
                yield
            for f in range(2, CT):
                ps = mps.tile([128, 512], F32, tag="mp", name=f"pj{f}")
                for ct in range(CT):
                    nc.tensor.matmul(
                        ps, wp[:, ct, f * 128:(f + 1) * 128], oT[ch][ct],
                        start=(ct == 0), stop=(ct == CT - 1))
                    if ct < CT - 1:
                        yield
                nc.vector.scalar_tensor_tensor(
                    out=x2T[f][:, cs], in0=ps,
                    scalar=pb_s[:, f:f + 1], in1=xoT[f][:, cs],
                    op0=ALU.add, op1=ALU.add)
                yield

        def ln2_quanta(ch, lw):
            cs = slice(ch * 512, (ch + 1) * 512)
            psum = mps.tile([1, 512], F32, tag="mp", name="psum")
            pssq = mps.tile([1, 512], F32, tag="mp", name="pssq")
            for ct in range(CT):
                sq = lw.tile([128, 512], BF16, tag="sq", bufs=1)
                nc.vector.tensor_mul(sq, x2T[ct][:, cs], x2T[ct][:, cs])
                nc.tensor.matmul(psum, ones_cb, x2T[ct][:, cs],
                                 start=(ct == 0), stop=(ct == CT - 1),
                                 skip_group_check=True)
                nc.tensor.matmul(pssq, ones_cb, sq,
                                 start=(ct == 0), stop=(ct == CT - 1),
                                 skip_group_check=True)
                yield
            # serial stats chain (sqrt lands at a head-pair boundary so the
            # one table round-trip hides under S/AV matmuls)
            mu = lw.tile([1, 512], F32, tag="mu", name=f"mu{ch}", bufs=1)
            nc.vector.tensor_scalar_mul(mu, psum, 1.0 / C)
            mu2 = lw.tile([1, 512], F32, tag="mu2", bufs=1)
            nc.vector.tensor_mul(mu2, mu, mu)
            var = lw.tile([1, 512], F32, tag="var", bufs=1)
            nc.vector.scalar_tensor_tensor(
                out=var, in0=pssq, scalar=1.0 / C, in1=mu2,
                op0=ALU.mult, op1=ALU.subtract)
            std = lw.tile([1, 512], F32, tag="mu2", name="std", bufs=1)
            nc.scalar.activation(out=std, in_=var, func=AF.Sqrt,
                                 bias=eps_t[0:1], scale=1.0)
            rstdf = lw.tile([1, 512], F32, tag="rstdf", bufs=1)
            nc.vector.reciprocal(out=rstdf, in_=std)
            nmr = lw.tile([1, 512], BF16, tag=f"nmr2_{ch}", name="nmr2",
                          bufs=1)
            nc.vector.scalar_tensor_tensor(
                out=nmr, in0=mu, scalar=-1.0, in1=rstdf,
                op0=ALU.mult, op1=ALU.mult)
            rstd = lw.tile([1, 512], BF16, tag=f"rstd2_{ch}", name="rstd2",
                           bufs=1)
            nc.vector.tensor_copy(out=rstd, in_=rstdf)
            ln2v[ch] = (rstd, nmr)
            yield

        def h2_quanta(ch, lw, h2_pool, bc_pool):
            """h2 = ((x2T*bc_r + bc_m)*g2 + b2): broadcast on gpsimd,
            affine on DVE (keeps ScalarE free for exps)."""
            cs = slice(ch * 512, (ch + 1) * 512)
            rstd, nmr = ln2v[ch]
            bc_r = bc_pool.tile([128, 512], BF16, tag="bc_r")
            nc.gpsimd.partition_broadcast(bc_r, rstd, channels=128)
            bc_m = bc_pool.tile([128, 512], BF16, tag="bc_m")
            nc.gpsimd.partition_broadcast(bc_m, nmr, channels=128)
            yield
            for ct in range(CT):
                t = lw.tile([128, 512], BF16, tag="h2tmp")
                nc.vector.tensor_mul(t, x2T[ct][:, cs], bc_r)
                nc.vector.tensor_add(t, t, bc_m)
                h2 = h2_pool.tile([128, 512], BF16,
                                  tag=f"h2_{ct}_{ch}", name=f"h2_{ct}_{ch}")
                nc.vector.tensor_scalar(
                    out=h2, in0=t, scalar1=g2_s[:, ct:ct + 1],
                    scalar2=b2_s[:, ct:ct + 1],
                    op0=ALU.mult, op1=ALU.add)
                h2c[ch][ct] = h2
                yield

        # fc1 matmuls only, staging pre-gelu h1 in bf16 (gelu deferred so
        # ScalarE stays on the exp table set during attention)
        def fc1_stage_quanta(ch, hb, w1t, gbuf, h1s):
            for ht in range(HQ):
                ps = mps.tile([128, 512], F32, tag="mp", name="f1")
                for ct in range(CT):
                    nc.tensor.matmul(
                        ps, w1t[:, ct, ht * 128:(ht + 1) * 128],
                        h2c[ch][ct],
                        start=(ct == 0), stop=(ct == CT - 1))
                    if ct < CT - 1:
                        yield
                h1 = gbuf.tile([128, 512], BF16, tag="gb",
                               name=f"h1_{ch}_{hb}_{ht}")
                nc.vector.tensor_copy(out=h1, in_=ps)
                h1s[ht] = h1
                yield

        def gelu_piece(ch, hb, gbuf, h1s, gs):
            for ht in range(HQ):
                g = gbuf.tile([128, 512], BF16, tag="gb",
                              name=f"g_{ch}_{hb}_{ht}")
                hti = hb * HQ + ht
                nc.scalar.activation(out=g, in_=h1s[ht], func=AF.Gelu,
                                     bias=f1b_s[:, hti:hti + 1], scale=1.0)
                gs[ht] = g

        # fc1 with gelu fused right off PSUM (post-attention phases)
        def fc1_full(ch, hb, w1t, gbuf, gs):
            for ht in range(HQ):
                ps = mps.tile([128, 512], F32, tag="mp", name="f1")
                for ct in range(CT):
                    nc.tensor.matmul(
                        ps, w1t[:, ct, ht * 128:(ht + 1) * 128],
                        h2c[ch][ct],
                        start=(ct == 0), stop=(ct == CT - 1))
                g = gbuf.tile([128, 512], BF16, tag="gb",
                              name=f"g_{ch}_{hb}_{ht}")
                hti = hb * HQ + ht
                nc.scalar.activation(out=g, in_=ps, func=AF.Gelu,
                                     bias=f1b_s[:, hti:hti + 1], scale=1.0)
                gs[ht] = g

        acc = {}

        def fc2_piece(ch, hb, w2t, gs, accp, osb):
            cs = slice(ch * 512, (ch + 1) * 512)
            for ft in range(CT):
                ps = mps.tile([128, 512], F32, tag="mp", name="f2")
                for ht in range(HQ):
                    nc.tensor.matmul(
                        ps, w2t[:, ht, ft * 128:(ft + 1) * 128], gs[ht],
                        start=(ht == 0), stop=(ht == HQ - 1))
                if hb == 0:
                    a = accp.tile([128, 512], BF16, tag=f"acc{ft}_{ch}",
                                  name=f"acc{ft}_{ch}")
                    acc[(ft, ch)] = a
                    nc.vector.scalar_tensor_tensor(
                        out=a, in0=ps,
                        scalar=f2b_s[:, ft:ft + 1],
                        in1=x2T[ft][:, cs],
                        op0=ALU.add, op1=ALU.add)
                else:
                    ot = osb.tile([128, 512], F32, tag="ot")
                    nc.vector.tensor_add(ot, ps, acc[(ft, ch)])
                    dma(out=outT[ft * 128:(ft + 1) * 128, cs], in_=ot)

        # =================== attention (both chunks, flat) ===================
        env = {}
        h1s0 = [None] * HQ  # staged pre-gelu fc1 outputs (ch0, hb0)
        HH = HID // 2

        def boundary(ch):
            if ch == 1:
                # hkvT/wq freed -> SBUF headroom for the MLP working set;
                # wproj + wfc1(h0) DMAs ride under chunk 1 (their quanta
                # start ~25us in).
                _hstk.close()
                env["wp_pool"] = pool(name="wproj", bufs=1, side="right")
                env["w1_pool"] = pool(name="wfc1", bufs=1, side="right")
                env["gbuf"] = pool(name="gbuf", bufs=13, side="right")
                env["h2a_pool"] = pool(name="h2a_sb", bufs=1, side="right")
                env["acc_pool"] = pool(name="acc_sb", bufs=1, side="right")
                env["bc_pool"] = pool(name="bc_sb", bufs=1, side="right")
                env["lw"] = pool(name="mlp_work", bufs=1, side="right")
                wp = env["wp_pool"].tile([128, CT, C], BF16, tag="wproj")
                dma(out=wp, in_=wprojT.rearrange("(ct p) f -> p ct f", p=128))
                env["wp"] = wp
                w1t0 = env["w1_pool"].tile([128, CT, HH], BF16, tag="wfc1",
                                           name="w1h0")
                dma(out=w1t0,
                    in_=wfc1T.rearrange("(ct p) f -> p ct f", p=128)
                    [:, :, 0:HH])
                env["w1t0"] = w1t0

        def mk_feed(ch):
            f = Feed()
            if ch == 0:
                for ft in (3, 4, 5):
                    for chk in range(4):
                        f.add(ft, k_chunk(ft, chk))
                    for chk in range(QCH):
                        f.add(ft, q_chunk(ft, chk))
            else:
                f.add(99, proj_quanta(0, env["wp"]))
                f.add(99, ln2_quanta(0, env["lw"]))
                f.add(99, h2_quanta(0, env["lw"], env["h2a_pool"],
                                    env["bc_pool"]))
                f.add(99, fc1_stage_quanta(0, 0, env["w1t0"], env["gbuf"],
                                           h1s0))
            return f

        attention_all(mk_feed, boundary)
        _stk.close()  # free qT/kT/vA/pt pools for the MLP tail
        wp, w1t0 = env["wp"], env["w1t0"]
        gbuf, lw = env["gbuf"], env["lw"]

        # =================== MLP tail ===================
        w2_pool = pool(name="wfc2", bufs=1, side="right")
        h2b_pool = pool(name="h2b_sb", bufs=1, side="right")
        accb_pool = pool(name="accb_sb", bufs=1, side="right")
        osb = pool(name="out_sb", bufs=2, side="right")
        w2t0 = w2_pool.tile([128, HQ, C], BF16, tag="wfc2", name="w2h0")
        dma(out=w2t0,
            in_=wfc2T.rearrange("(ht p) f -> p ht f", p=128)[:, 0:HQ, :])
        w1t1 = env["w1_pool"].tile([128, CT, HH], BF16, tag="wfc1b",
                                   name="w1h1")
        dma(out=w1t1,
            in_=wfc1T.rearrange("(ct p) f -> p ct f", p=128)[:, :, HH:HID])
        w2t1 = w2_pool.tile([128, HQ, C], BF16, tag="wfc2b", name="w2h1")
        dma(out=w2t1,
            in_=wfc2T.rearrange("(ht p) f -> p ht f", p=128)[:, HQ:HT, :])

        # gelu00 + fc2(0,0) first: fc2's f0 matmuls trail the gelu stream
        # (only tail work with no dependence on chunk1's oT), then proj(1)
        # once oT(1) lands, then the rest at full PE rate.
        g00 = [None] * HQ
        gelu_piece(0, 0, gbuf, h1s0, g00)
        fc2_piece(0, 0, w2t0, g00, env["acc_pool"], None)
        run_gen(proj_quanta(1, wp))
        run_gen(ln2_quanta(1, lw))
        g01 = [None] * HQ
        fc1_full(0, 1, w1t1, gbuf, g01)
        run_gen(h2_quanta(1, lw, h2b_pool, env["bc_pool"]))
        fc2_piece(0, 1, w2t1, g01, None, osb)
        g10 = [None] * HQ
        fc1_full(1, 0, w1t0, gbuf, g10)
        fc2_piece(1, 0, w2t0, g10, accb_pool, None)
        g11 = [None] * HQ
        fc1_full(1, 1, w1t1, gbuf, g11)
        fc2_piece(1, 1, w2t1, g11, None, osb)


def kernel(**inputs):
    global _NC_CACHE, LAST_RESULTS
    import os
    ins = {k: np.ascontiguousarray(np.asarray(v, dtype=np.float32))
           for k, v in inputs.items()}
    if _NC_CACHE is None:
        _NC_CACHE = build_program()
    nc = _NC_CACHE

    import ml_dtypes
    bf = ml_dtypes.bfloat16

    def col6(v):  # [768] -> [128, 6] matching "(ct p) -> p ct"
        return np.ascontiguousarray(v.reshape(6, 128).T)

    cvec = np.concatenate(
        [col6(ins["ln1_g"]), col6(ins["ln1_b"]), col6(ins["ln2_g"]),
         col6(ins["ln2_b"]), col6(ins["proj_b"]), col6(ins["fc2_b"]),
         np.ascontiguousarray(ins["fc1_b"].reshape(HT, 128).T)],
        axis=1).astype(np.float32)

    shared = {
        "wqkvT": np.ascontiguousarray(ins["qkv_w"].T.astype(bf)),
        "wprojT": np.ascontiguousarray(ins["proj_w"].T.astype(bf)),
        "wfc1T": np.ascontiguousarray(ins["fc1_w"].T.astype(bf)),
        "wfc2T": np.ascontiguousarray(ins["fc2_w"].T.astype(bf)),
        "cvec": np.ascontiguousarray(cvec),
    }
    in_maps = []
    for s in range(NCORES):
        b, half = s // 2, s % 2
        m = dict(shared)
        m["xbT"] = np.ascontiguousarray(
            np.roll(ins["x"][b], -half * NO, axis=0).T.astype(bf))
        in_maps.append(m)

    trace = bool(int(os.environ.get("KBENCH_TRACE", "0")))
    LAST_RESULTS = run_bass_kernel_spmd(
        nc, in_maps, core_ids=list(range(NCORES)), trace=trace)
    out = np.empty((B, N, C), np.float32)
    for s in range(NCORES):
        b, half = s // 2, s % 2
        out[b, half * NO:(half + 1) * NO, :] = LAST_RESULTS.results[s]["outT"].T
    return out


# revision 39
# speedup vs baseline: 1.0708x; 1.0708x over previous
"""Trainium2 Bass kernel for a dense transformer block (B=4, N=2048, C=768,
H=12, D=64, HID=3072), sharded over 8 NeuronCores.

Sharding: token-split, no collectives. Core s handles batch b = s//2,
sequence half = s%2 (1024 tokens). Each core receives its batch element's
full 2048-token x (rolled so its own tokens are rows 0..1023), computes
K/V over all 2048 tokens (redundantly with its pair core), and produces
the output for its own 1024 tokens. Host gathers/transposes.

v3 schedule (trace-driven rewrite of v2):
- era1 interleaves per 512-token group: x DMA -> raw-x^T transposes (PE
  work with no LN dependency) -> LN1 -> h^T transposes -> V -> K/Q
  chunks, so PE/Scalar/DVE overlap from t~2us. Consts land in ONE
  batched DMA (cvec).
- attention is ScalarE(exp)-bound; PE fillers are fine-grained QUANTA
  (1 matmul each) pulled one-per-step inside the S->exp->AV pipeline,
  with per-head-pair deadlines (kq ft3-5 feed chunk0's own later head
  pairs; proj/LN2/h2/fc1-staging of chunk0 run during chunk1).
- softmax denominators: batched reciprocal split in two halves per
  chunk so oT normalization starts 3 head-pairs early and proj quanta
  are ready at chunk1 start / tail start.
- all rstd use Exp(-0.5*Ln(var+eps)) -- same activation table set as
  the softmax exp (natural_log_exp_and_others); Gelu is the only other
  set, so 2 ACT table loads total instead of 12.
- weights prefetch: wproj+wfc1(h0) at chunk0 start, wfc1(h1) at chunk1
  start, wfc2(h1) at tail start -- tail never waits on DMA.
- tail order: proj(1) -> ln2(1)mm -> fc2(0,0) -> fc1(0,1) -> fc2(0,1)
  -> fc1(1,0) -> fc2(1,0) -> fc1(1,1) -> fc2(1,1), gelu fused from
  PSUM, so the LN2(1) serial chain hides under fc2/fc1 matmuls.
"""

from contextlib import ExitStack

import numpy as np

import concourse.bass as bass
import concourse.mybir as mybir
import concourse.tile as tile
from concourse import bacc
from concourse.bass_utils import run_bass_kernel_spmd
from concourse.masks import make_identity

F32 = mybir.dt.float32
BF16 = mybir.dt.bfloat16
AF = mybir.ActivationFunctionType
ALU = mybir.AluOpType

B, N, C = 4, 2048, 768
H, D = 12, 64
HID = 3072
EPS = 1e-5
NCORES = 8
NO = 1024  # tokens owned per core
NKV = 2048  # key/value tokens per core
CT = C // 128  # 6 feature tiles
HT = HID // 128  # 24 hidden tiles
HQ = HT // 2  # 12 hidden tiles per half
KT = NKV // 128  # 16 kv token tiles
QCH = NO // 512  # 2 query chunks of 512
ISCALE = 1.0 / np.sqrt(D)
SKEW = 2  # S->AV software-pipeline depth (in nt steps)
NCV = 36 + HT  # cvec columns: g1,b1,g2,b2,pb,f2b (6 ea) + f1b (24)

LAST_RESULTS = None
_NC_CACHE = None


def build_program(repeats=1):
    nc = bacc.Bacc(trn_type="TRN2", target_bir_lowering=False, num_devices=NCORES)

    xb = nc.dram_tensor("xb", [NKV, C], F32, kind="ExternalInput").ap()
    wqkvT = nc.dram_tensor("wqkvT", [C, 3 * C], BF16, kind="ExternalInput").ap()
    wprojT = nc.dram_tensor("wprojT", [C, C], BF16, kind="ExternalInput").ap()
    wfc1T = nc.dram_tensor("wfc1T", [C, HID], BF16, kind="ExternalInput").ap()
    wfc2T = nc.dram_tensor("wfc2T", [HID, C], BF16, kind="ExternalInput").ap()
    cvec = nc.dram_tensor("cvec", [128, NCV], F32, kind="ExternalInput").ap()
    outT = nc.dram_tensor("outT", [C, NO], F32, kind="ExternalOutput").ap()

    with tile.TileContext(nc) as tc:
        for _ in range(repeats):
            emit(nc, tc, xb, wqkvT, wprojT, wfc1T, wfc2T, cvec, outT)
    nc.compile()
    return nc


def emit(nc, tc, xb, wqkvT, wprojT, wfc1T, wfc2T, cvec, outT):
    dma = nc.sync.dma_start
    qs = (slice(0, 64), slice(64, 128))

    with ExitStack() as _st:
        def pool(**kw):
            return _st.enter_context(tc.tile_pool(**kw))

        consts = pool(name="consts", bufs=1)
        x2T_pool = pool(name="x2T", bufs=1)
        xoT_pool = pool(name="xoT", bufs=1)

        # ---- constants: one batched DMA for every small vector ----
        cv = consts.tile([128, NCV], F32, tag="cv")
        dma(out=cv, in_=cvec)
        g1_s = cv[:, 0:6]
        b1_s = cv[:, 6:12]
        g2_s = cv[:, 12:18]
        b2_s = cv[:, 18:24]
        pb_s = cv[:, 24:30]
        f2b_s = cv[:, 30:36]
        f1b_s = cv[:, 36:36 + HT]

        ident = consts.tile([128, 128], F32, tag="ident")
        make_identity(nc, ident)
        ident_bf = consts.tile([128, 128], BF16, tag="ident_bf")
        make_identity(nc, ident_bf)
        ones_f32 = consts.tile([128, 1], F32, tag="ones_f32")
        nc.vector.memset(ones_f32, 1.0)
        ones_cb = consts.tile([128, 1], BF16, tag="ones_cb")
        nc.scalar.activation(out=ones_cb, in_=ones_f32, func=AF.Copy)
        eps_t = consts.tile([128, 1], F32, tag="eps")
        nc.vector.memset(eps_t, EPS)

        # residual streams (bf16 to fit SBUF; rel-err budget is 2e-2).
        # xoT holds the host-transposed raw x^T for the OWN tokens -- it
        # doubles as the residual stream (no PE transposes anywhere).
        x2T = [x2T_pool.tile([128, NO], BF16, tag=f"x2T{ct}", name=f"x2T{ct}")
               for ct in range(CT)]
        xoT = [xoT_pool.tile([128, NO], BF16, tag=f"xoT{ct}", name=f"xoT{ct}")
               for ct in range(CT)]

        # ---- PSUM pools: sps 4 banks, ops 2, mps 2 (8 total) ----
        sps = pool(name="s_psum", bufs=2, space="PSUM")
        ops = pool(name="o_psum", bufs=2, space="PSUM")
        mps = pool(name="m_psum", bufs=2, space="PSUM")

        oU_pool = pool(name="oU", bufs=12)
        oT_pool = pool(name="oT", bufs=6)
        den_pool = pool(name="den", bufs=1)
        asm = pool(name="asm", bufs=2)

        _stk = ExitStack()  # attention working set: freed at tail

        def kpool(**kw):
            return _stk.enter_context(tc.tile_pool(**kw))

        qT_pool = kpool(name="qT", bufs=1)
        kT_pool = kpool(name="kT", bufs=1)
        vA_pool = kpool(name="vA", bufs=1)
        p_sb = kpool(name="p_sb", bufs=4)
        qT = [qT_pool.tile([128, NO], BF16, tag=f"qT{ct}", name=f"qT{ct}")
              for ct in range(CT)]
        kT = [kT_pool.tile([128, NKV], BF16, tag=f"kT{ct}", name=f"kT{ct}")
              for ct in range(CT)]
        vA = [vA_pool.tile([128, H, D + 1], BF16, tag=f"vA{nt}",
                           name=f"vA{nt}") for nt in range(KT)]

        dent = [None, None]
        oT = [[None] * CT for _ in range(QCH)]
        oU_all = [[None] * H for _ in range(QCH)]
        ln2v = [None, None]
        h2c = [[None] * CT for _ in range(QCH)]

        _hstk = ExitStack()  # hkvT/wq: freed at chunk0|chunk1 boundary
        hkvT_pool = _hstk.enter_context(tc.tile_pool(name="hkvT", bufs=1))
        wq_pool = _hstk.enter_context(tc.tile_pool(name="wqkv", bufs=1))
        hkvT = [hkvT_pool.tile([128, NKV], BF16, tag=f"hkvT{ct}",
                               name=f"hkvT{ct}") for ct in range(CT)]
        wq = wq_pool.tile([128, CT, 3 * C], BF16, tag="wqkv")
        # wq rides the scalar-engine DMA path in K,V,Q column order so the
        # sync queue stays clear for x-tile prefetch and each slice lands
        # just before its first consumer in group 0.
        _wq_src = wqkvT.rearrange("(ct p) f -> p ct f", p=128)
        nc.scalar.dma_start(out=wq[:, :, C:2 * C], in_=_wq_src[:, :, C:2 * C])
        nc.scalar.dma_start(out=wq[:, :, 2 * C:3 * C],
                            in_=_wq_src[:, :, 2 * C:3 * C])
        nc.scalar.dma_start(out=wq[:, :, 0:C], in_=_wq_src[:, :, 0:C])

        # =================== era1: per-group interleave ===================
        def v_tile(nt):
            """V for one kv token tile -> vA[nt] (sps psum, 3 banks used)."""
            psAB = sps.tile([128, 1024], F32, tag="ps", name=f"psAB{nt}")
            for ct in range(CT):
                hk = hkvT[ct][:, nt * 128:(nt + 1) * 128]
                nc.tensor.matmul(psAB[:, 0:512], hk, wq[:, ct, 2 * C:2 * C + 512],
                                 start=(ct == 0), stop=(ct == CT - 1))
                nc.tensor.matmul(psAB[:, 512:768], hk,
                                 wq[:, ct, 2 * C + 512:3 * C],
                                 start=(ct == 0), stop=(ct == CT - 1))
            nc.vector.tensor_copy(
                out=vA[nt][:, 0:8, 0:D],
                in_=psAB[:, 0:512].rearrange("p (h d) -> p h d", d=D))
            nc.vector.tensor_copy(
                out=vA[nt][:, 8:12, 0:D],
                in_=psAB[:, 512:768].rearrange("p (h d) -> p h d", d=D))
            nc.vector.memset(vA[nt][:, :, D:D + 1], 1.0)

        def k_chunk(ft, chk):
            ps = mps.tile([128, 512], F32, tag="mp", name=f"kc{ft}_{chk}")
            for ct in range(CT):
                nc.tensor.matmul(
                    ps, wq[:, ct, C + ft * 128:C + (ft + 1) * 128],
                    hkvT[ct][:, chk * 512:(chk + 1) * 512],
                    start=(ct == 0), stop=(ct == CT - 1))
                if ct < CT - 1:
                    yield
            nc.vector.tensor_copy(
                out=kT[ft][:, chk * 512:(chk + 1) * 512], in_=ps)
            yield

        def q_chunk(ft, chk):
            ps = mps.tile([128, 512], F32, tag="mp", name=f"qc{ft}_{chk}")
            for ct in range(CT):
                nc.tensor.matmul(
                    ps, wq[:, ct, ft * 128:(ft + 1) * 128],
                    hkvT[ct][:, chk * 512:(chk + 1) * 512],
                    start=(ct == 0), stop=(ct == CT - 1))
                if ct < CT - 1:
                    yield
            nc.vector.tensor_copy(
                out=qT[ft][:, chk * 512:(chk + 1) * 512], in_=ps)
            yield

        def run_gen(g):
            for _ in g:
                pass

        with (
            tc.tile_pool(name="xw", bufs=2) as xpool,
            tc.tile_pool(name="ln1_stat", bufs=6) as lstat,
        ):
            for g in range(KT // 4):  # groups of 4 token tiles (512 tokens)
                xts, xcs = [], []
                for j in range(4):
                    nt = 4 * g + j
                    xt = xpool.tile([128, C], F32, tag=f"xt{j}", name=f"xt{j}")
                    dma(out=xt, in_=xb[nt * 128:(nt + 1) * 128, :])
                    st = lstat.tile([128, 3, 6], F32, tag="st")
                    xg = xt.rearrange("p (s d) -> p s d", s=3)
                    for s in range(3):
                        nc.vector.bn_stats(out=st[:, s], in_=xg[:, s])
                    mv = lstat.tile([128, 2], F32, tag="mv")
                    nc.vector.bn_aggr(out=mv, in_=st)
                    rstd = lstat.tile([128, 1], F32, tag="rstd")
                    nc.scalar.activation(out=rstd, in_=mv[:, 1:2],
                                         func=AF.Sqrt, bias=eps_t, scale=1.0)
                    nc.vector.reciprocal(out=rstd, in_=rstd)
                    nmr = lstat.tile([128, 1], F32, tag="nmr")
                    nc.vector.tensor_scalar(out=nmr, in0=mv[:, 0:1],
                                            scalar1=-1.0, scalar2=rstd,
                                            op0=ALU.mult, op1=ALU.mult)
                    xc = xpool.tile([128, C], BF16, tag=f"xc{j}",
                                    name=f"xc{j}", bufs=1)
                    nc.scalar.activation(out=xc, in_=xt, func=AF.Identity,
                                         scale=rstd, bias=nmr)
                    xts.append(xt)
                    xcs.append(xc)
                # interleave raw-x^T (DVE drain) and h^T (ACT drain)
                # transposes so the two drain engines run concurrently.
                for ct in range(CT):
                    if g < NO // 512:
                        ps32 = mps.tile([128, 512], F32, tag="mp",
                                        name="ps32")
                        for j in range(4):
                            nc.tensor.transpose(
                                ps32[:, j * 128:(j + 1) * 128],
                                xts[j][:, ct * 128:(ct + 1) * 128], ident)
                        nc.vector.tensor_copy(
                            out=xoT[ct][:, g * 512:(g + 1) * 512], in_=ps32)
                    ps = mps.tile([128, 512], BF16, tag="mp", name="pst")
                    for j in range(4):
                        nc.tensor.transpose(
                            ps[:, j * 128:(j + 1) * 128],
                            xcs[j][:, ct * 128:(ct + 1) * 128], ident_bf)
                    nc.scalar.activation(
                        out=hkvT[ct][:, g * 512:(g + 1) * 512],
                        in_=ps, func=AF.Identity,
                        scale=g1_s[:, ct:ct + 1], bias=b1_s[:, ct:ct + 1])
                for ft in range(3):  # ft0-2 K here; ft3-5 are attn fillers
                    run_gen(k_chunk(ft, g))
                for j in range(4):
                    v_tile(4 * g + j)
                if g < QCH:
                    for ft in range(3):
                        run_gen(q_chunk(ft, g))

        # =================== attention machinery ===================
        def ot_norm_piece(ch, hp, rec, base):
            """Normalize one head pair's output into oT[ch][hp]."""
            t = oT_pool.tile([128, 512], BF16, tag="oT",
                             name=f"oT{ch}_{hp}")
            for i in range(2):
                r = 2 * hp + i
                rb = asm.tile([1, 512], BF16, tag="rb")
                # gpsimd-queue DMA: gpsimd is idle, and this never
                # head-of-line blocks the sync queue's big transfers
                nc.gpsimd.dma_start(out=rb, in_=rec[r - base:r - base + 1, :])
                vb = asm.tile([D, 512], BF16, tag="vb")
                nc.gpsimd.partition_broadcast(vb, rb, channels=D)
                nc.vector.tensor_mul(t[qs[i], :], oU_all[ch][r], vb)
            oT[ch][hp] = t

        def rec_quarter(ch, q):
            """Batched reciprocal of 4 denominator rows (heads 4q..4q+3),
            then normalize head pairs 2q and 2q+1 -- runs as soon as each
            third of a chunk's denominators lands so oT is ready early."""
            rec = dent[ch][q]
            with nc.allow_low_precision(reason="softmax denom in bf16"):
                nc.vector.reciprocal(out=rec, in_=rec)
            for hp in (2 * q, 2 * q + 1):
                ot_norm_piece(ch, hp, rec, 4 * q)

        class Feed:
            """Deadline-ordered filler quanta. pull() emits one quantum;
            flush(hp) force-emits every generator due before head pair hp
            (PE is in-order, so a quantum consumed by hp's matmuls must be
            emitted before them)."""

            def __init__(self):
                self.items = []

            def add(self, deadline, gen):
                self.items.append([deadline, gen])

            def pull(self):
                for it in self.items:
                    if it[1] is not None:
                        try:
                            next(it[1])
                            return True
                        except StopIteration:
                            it[1] = None
                return False

            def flush(self, hp):
                for it in self.items:
                    if it[1] is not None and it[0] <= hp:
                        for _ in it[1]:
                            pass
                        it[1] = None

            def flush_all(self):
                self.flush(10 ** 9)

        def attention_all(feed_of, boundary_hook):
            """Both chunks as ONE continuously-skewed (ch, hp, nt) stream:
            the next head pair's S matmuls interleave with the previous
            pair's AV tail, so the exp stream never pauses at boundaries.
            feed_of(ch) -> Feed; boundary_hook(ch) fires before chunk ch's
            first S (pool swaps / weight prefetch / feed construction)."""
            seq = [(ch, hp, nt) for ch in range(QCH)
                   for hp in range(CT) for nt in range(KT)]
            po_cur = {}
            pts = {}
            feed = None
            for idx in range(len(seq) + SKEW):
                if idx < len(seq):
                    ch, hp, nt = seq[idx]
                    if nt == 0:
                        if hp == 0:
                            if feed is not None:
                                feed.flush_all()
                            boundary_hook(ch)
                            feed = feed_of(ch)
                            dent[ch] = [
                                den_pool.tile([4, 512], BF16, tag=f"dent{h}",
                                              name=f"dent{ch}_{h}")
                                for h in range(3)]
                        feed.flush(hp)
                    qch = slice(ch * 512, (ch + 1) * 512)
                    ps2 = sps.tile([128, 1024], F32, tag="ps")
                    for i in range(2):
                        nc.tensor.matmul(
                            ps2[:, i * 512:(i + 1) * 512],
                            kT[hp][qs[i], nt * 128:(nt + 1) * 128],
                            qT[hp][qs[i], qch],
                            start=True, stop=True,
                            tile_position=(64 * i, 0))
                    pt2 = p_sb.tile([128, 1024], BF16, tag="pt")
                    nc.scalar.activation(out=pt2, in_=ps2,
                                         func=AF.Exp, scale=ISCALE)
                    pts[(ch, hp, nt)] = pt2
                if idx >= SKEW:
                    ch_a, hp_a, m = seq[idx - SKEW]
                    pt2 = pts.pop((ch_a, hp_a, m))
                    if m == 0:
                        po_cur[(ch_a, hp_a)] = [
                            ops.tile([D + 1, 512], F32, tag="po",
                                     name=f"po{ch_a}_{hp_a}_{i}")
                            for i in range(2)]
                    po = po_cur[(ch_a, hp_a)]
                    for i in range(2):
                        nc.tensor.matmul(
                            po[i], vA[m][:, 2 * hp_a + i, :],
                            pt2[:, i * 512:(i + 1) * 512],
                            start=(m == 0), stop=(m == KT - 1),
                            skip_group_check=True)
                    if m == KT - 1:
                        # drain: unnormalized O^T halves + denominator row
                        for i in range(2):
                            r = 2 * hp_a + i
                            oU = oU_pool.tile([D, 512], BF16, tag="oU",
                                              name=f"oU{ch_a}_{r}")
                            nc.vector.tensor_copy(out=oU, in_=po[i][0:D, :])
                            oU_all[ch_a][r] = oU
                            dt = asm.tile([1, 512], BF16, tag="dtmp")
                            nc.vector.tensor_copy(out=dt, in_=po[i][D:D + 1, :])
                            nc.gpsimd.dma_start(
                                out=dent[ch_a][r // 4][(r % 4):(r % 4) + 1, :],
                                in_=dt)
                        del po_cur[(ch_a, hp_a)]
                        if hp_a % 2 == 1:
                            rec_quarter(ch_a, hp_a // 2)
                # filler pulls: ~1.5/step keeps PE full without starving
                # the exp stream, and drains inventory before chunk end
                feed.pull()
                if idx % 2 == 0:
                    feed.pull()

        # ------------- proj / LN2 / MLP building blocks -------------
        def proj_quanta(ch, wp):
            """proj + x2 residual for chunk ch. f0/f1 start on the first
            five oT tiles so quanta are ready before oT[5] lands."""
            cs = slice(ch * 512, (ch + 1) * 512)
            ps01 = []
            for f in range(2):
                ps = mps.tile([128, 512], F32, tag="mp", name=f"pj{f}")
                ps01.append(ps)
                for ct in range(CT - 1):
                    nc.tensor.matmul(
                        ps, wp[:, ct, f * 128:(f + 1) * 128], oT[ch][ct],
                        start=(ct == 0), stop=False)
                    yield
            for f in range(2):
                nc.tensor.matmul(
                    ps01[f], wp[:, CT - 1, f * 128:(f + 1) * 128],
                    oT[ch][CT - 1], start=False, stop=True)
                nc.vector.scalar_tensor_tensor(
                    out=x2T[f][:, cs], in0=ps01[f],
                    scalar=pb_s[:, f:f + 1], in1=xoT[f][:, cs],
                    op0=ALU.add, op1=ALU.add)
                yield
            for f in range(2, CT):
                ps = mps.tile([128, 512], F32, tag="mp", name=f"pj{f}")
                for ct in range(CT):
                    nc.tensor.matmul(
                        ps, wp[:, ct, f * 128:(f + 1) * 128], oT[ch][ct],
                        start=(ct == 0), stop=(ct == CT - 1))
                    if ct < CT - 1:
                        yield
                nc.vector.scalar_tensor_tensor(
                    out=x2T[f][:, cs], in0=ps,
                    scalar=pb_s[:, f:f + 1], in1=xoT[f][:, cs],
                    op0=ALU.add, op1=ALU.add)
                yield

        def ln2_quanta(ch, lw):
            cs = slice(ch * 512, (ch + 1) * 512)
            psum = mps.tile([1, 512], F32, tag="mp", name="psum")
            pssq = mps.tile([1, 512], F32, tag="mp", name="pssq")
            for ct in range(CT):
                sq = lw.tile([128, 512], BF16, tag="sq", bufs=1)
                nc.vector.tensor_mul(sq, x2T[ct][:, cs], x2T[ct][:, cs])
                nc.tensor.matmul(psum, ones_cb, x2T[ct][:, cs],
                                 start=(ct == 0), stop=(ct == CT - 1),
                                 skip_group_check=True)
                nc.tensor.matmul(pssq, ones_cb, sq,
                                 start=(ct == 0), stop=(ct == CT - 1),
                                 skip_group_check=True)
                yield
            # serial stats chain (sqrt lands at a head-pair boundary so the
            # one table round-trip hides under S/AV matmuls)
            mu = lw.tile([1, 512], F32, tag="mu", name=f"mu{ch}", bufs=1)
            nc.vector.tensor_scalar_mul(mu, psum, 1.0 / C)
            mu2 = lw.tile([1, 512], F32, tag="mu2", bufs=1)
            nc.vector.tensor_mul(mu2, mu, mu)
            var = lw.tile([1, 512], F32, tag="var", bufs=1)
            nc.vector.scalar_tensor_tensor(
                out=var, in0=pssq, scalar=1.0 / C, in1=mu2,
                op0=ALU.mult, op1=ALU.subtract)
            std = lw.tile([1, 512], F32, tag="mu2", name="std", bufs=1)
            nc.scalar.activation(out=std, in_=var, func=AF.Sqrt,
                                 bias=eps_t[0:1], scale=1.0)
            rstdf = lw.tile([1, 512], F32, tag="rstdf", bufs=1)
            nc.vector.reciprocal(out=rstdf, in_=std)
            nmr = lw.tile([1, 512], BF16, tag=f"nmr2_{ch}", name="nmr2",
                          bufs=1)
            nc.vector.scalar_tensor_tensor(
                out=nmr, in0=mu, scalar=-1.0, in1=rstdf,
                op0=ALU.mult, op1=ALU.mult)
            rstd = lw.tile([1, 512], BF16, tag=f"rstd2_{ch}", name="rstd2",
                           bufs=1)
            nc.vector.tensor_copy(out=rstd, in_=rstdf)
            ln2v[ch] = (rstd, nmr)
            yield

        def h2_quanta(ch, lw, h2_pool, bc_pool):
            """h2 = ((x2T*bc_r + bc_m)*g2 + b2): broadcast on gpsimd,
            affine on DVE (keeps ScalarE free for exps)."""
            cs = slice(ch * 512, (ch + 1) * 512)
            rstd, nmr = ln2v[ch]
            bc_r = bc_pool.tile([128, 512], BF16, tag="bc_r")
            nc.gpsimd.partition_broadcast(bc_r, rstd, channels=128)
            bc_m = bc_pool.tile([128, 512], BF16, tag="bc_m")
            nc.gpsimd.partition_broadcast(bc_m, nmr, channels=128)
            yield
            for ct in range(CT):
                t = lw.tile([128, 512], BF16, tag="h2tmp")
                nc.vector.tensor_mul(t, x2T[ct][:, cs], bc_r)
                nc.vector.tensor_add(t, t, bc_m)
                h2 = h2_pool.tile([128, 512], BF16,
                                  tag=f"h2_{ct}_{ch}", name=f"h2_{ct}_{ch}")
                nc.vector.tensor_scalar(
                    out=h2, in0=t, scalar1=g2_s[:, ct:ct + 1],
                    scalar2=b2_s[:, ct:ct + 1],
                    op0=ALU.mult, op1=ALU.add)
                h2c[ch][ct] = h2
                yield

        # fc1 matmuls only, staging pre-gelu h1 in bf16 (gelu deferred so
        # ScalarE stays on the exp table set during attention)
        def fc1_stage_quanta(ch, hb, w1t, gbuf, h1s):
            for ht in range(HQ):
                ps = mps.tile([128, 512], F32, tag="mp", name="f1")
                for ct in range(CT):
                    nc.tensor.matmul(
                        ps, w1t[:, ct, ht * 128:(ht + 1) * 128],
                        h2c[ch][ct],
                        start=(ct == 0), stop=(ct == CT - 1))
                    if ct < CT - 1:
                        yield
                h1 = gbuf.tile([128, 512], BF16, tag="gb",
                               name=f"h1_{ch}_{hb}_{ht}")
                nc.vector.tensor_copy(out=h1, in_=ps)
                h1s[ht] = h1
                yield

        def gelu_piece(ch, hb, gbuf, h1s, gs):
            for ht in range(HQ):
                g = gbuf.tile([128, 512], BF16, tag="gb",
                              name=f"g_{ch}_{hb}_{ht}")
                hti = hb * HQ + ht
                nc.scalar.activation(out=g, in_=h1s[ht], func=AF.Gelu,
                                     bias=f1b_s[:, hti:hti + 1], scale=1.0)
                gs[ht] = g

        # fc1 with gelu fused right off PSUM (post-attention phases)
        def fc1_full(ch, hb, w1t, gbuf, gs):
            for ht in range(HQ):
                ps = mps.tile([128, 512], F32, tag="mp", name="f1")
                for ct in range(CT):
                    nc.tensor.matmul(
                        ps, w1t[:, ct, ht * 128:(ht + 1) * 128],
                        h2c[ch][ct],
                        start=(ct == 0), stop=(ct == CT - 1))
                g = gbuf.tile([128, 512], BF16, tag="gb",
                              name=f"g_{ch}_{hb}_{ht}")
                hti = hb * HQ + ht
                nc.scalar.activation(out=g, in_=ps, func=AF.Gelu,
                                     bias=f1b_s[:, hti:hti + 1], scale=1.0)
                gs[ht] = g

        acc = {}

        def fc2_piece(ch, hb, w2t, gs, accp, osb):
            cs = slice(ch * 512, (ch + 1) * 512)
            for ft in range(CT):
                ps = mps.tile([128, 512], F32, tag="mp", name="f2")
                for ht in range(HQ):
                    nc.tensor.matmul(
                        ps, w2t[:, ht, ft * 128:(ft + 1) * 128], gs[ht],
                        start=(ht == 0), stop=(ht == HQ - 1))
                if hb == 0:
                    a = accp.tile([128, 512], BF16, tag=f"acc{ft}_{ch}",
                                  name=f"acc{ft}_{ch}")
                    acc[(ft, ch)] = a
                    nc.vector.scalar_tensor_tensor(
                        out=a, in0=ps,
                        scalar=f2b_s[:, ft:ft + 1],
                        in1=x2T[ft][:, cs],
                        op0=ALU.add, op1=ALU.add)
                else:
                    ot = osb.tile([128, 512], F32, tag="ot")
                    nc.vector.tensor_add(ot, ps, acc[(ft, ch)])
                    dma(out=outT[ft * 128:(ft + 1) * 128, cs], in_=ot)

        # =================== attention (both chunks, flat) ===================
        env = {}
        h1s0 = [None] * HQ  # staged pre-gelu fc1 outputs (ch0, hb0)
        HH = HID // 2

        def boundary(ch):
            if ch == 1:
                # hkvT/wq freed -> SBUF headroom for the MLP working set;
                # wproj + wfc1(h0) DMAs ride under chunk 1 (their quanta
                # start ~25us in).
                _hstk.close()
                env["wp_pool"] = pool(name="wproj", bufs=1, side="right")
                env["w1_pool"] = pool(name="wfc1", bufs=1, side="right")
                env["gbuf"] = pool(name="gbuf", bufs=13, side="right")
                env["h2a_pool"] = pool(name="h2a_sb", bufs=1, side="right")
                env["acc_pool"] = pool(name="acc_sb", bufs=1, side="right")
                env["bc_pool"] = pool(name="bc_sb", bufs=1, side="right")
                env["lw"] = pool(name="mlp_work", bufs=1, side="right")
                wp = env["wp_pool"].tile([128, CT, C], BF16, tag="wproj")
                dma(out=wp, in_=wprojT.rearrange("(ct p) f -> p ct f", p=128))
                env["wp"] = wp
                w1t0 = env["w1_pool"].tile([128, CT, HH], BF16, tag="wfc1",
                                           name="w1h0")
                dma(out=w1t0,
                    in_=wfc1T.rearrange("(ct p) f -> p ct f", p=128)
                    [:, :, 0:HH])
                env["w1t0"] = w1t0

        def mk_feed(ch):
            f = Feed()
            if ch == 0:
                for ft in (3, 4, 5):
                    for chk in range(4):
                        f.add(ft, k_chunk(ft, chk))
                    for chk in range(QCH):
                        f.add(ft, q_chunk(ft, chk))
            else:
                f.add(99, proj_quanta(0, env["wp"]))
                f.add(99, ln2_quanta(0, env["lw"]))
                f.add(99, h2_quanta(0, env["lw"], env["h2a_pool"],
                                    env["bc_pool"]))
                f.add(99, fc1_stage_quanta(0, 0, env["w1t0"], env["gbuf"],
                                           h1s0))
            return f

        attention_all(mk_feed, boundary)
        _stk.close()  # free qT/kT/vA/pt pools for the MLP tail
        wp, w1t0 = env["wp"], env["w1t0"]
        gbuf, lw = env["gbuf"], env["lw"]

        # =================== MLP tail ===================
        w2_pool = pool(name="wfc2", bufs=1, side="right")
        h2b_pool = pool(name="h2b_sb", bufs=1, side="right")
        accb_pool = pool(name="accb_sb", bufs=1, side="right")
        osb = pool(name="out_sb", bufs=2, side="right")
        w2t0 = w2_pool.tile([128, HQ, C], BF16, tag="wfc2", name="w2h0")
        dma(out=w2t0,
            in_=wfc2T.rearrange("(ht p) f -> p ht f", p=128)[:, 0:HQ, :])
        w1t1 = env["w1_pool"].tile([128, CT, HH], BF16, tag="wfc1b",
                                   name="w1h1")
        dma(out=w1t1,
            in_=wfc1T.rearrange("(ct p) f -> p ct f", p=128)[:, :, HH:HID])
        w2t1 = w2_pool.tile([128, HQ, C], BF16, tag="wfc2b", name="w2h1")
        dma(out=w2t1,
            in_=wfc2T.rearrange("(ht p) f -> p ht f", p=128)[:, HQ:HT, :])

        # gelu00 + fc2(0,0) first: fc2's f0 matmuls trail the gelu stream
        # (only tail work with no dependence on chunk1's oT), then proj(1)
        # once oT(1) lands, then the rest at full PE rate.
        g00 = [None] * HQ
        gelu_piece(0, 0, gbuf, h1s0, g00)
        fc2_piece(0, 0, w2t0, g00, env["acc_pool"], None)
        run_gen(proj_quanta(1, wp))
        run_gen(ln2_quanta(1, lw))
        g01 = [None] * HQ
        fc1_full(0, 1, w1t1, gbuf, g01)
        run_gen(h2_quanta(1, lw, h2b_pool, env["bc_pool"]))
        fc2_piece(0, 1, w2t1, g01, None, osb)
        g10 = [None] * HQ
        fc1_full(1, 0, w1t0, gbuf, g10)
        fc2_piece(1, 0, w2t0, g10, accb_pool, None)
        g11 = [None] * HQ
        fc1_full(1, 1, w1t1, gbuf, g11)
        fc2_piece(1, 1, w2t1, g11, None, osb)


def kernel(**inputs):
    global _NC_CACHE, LAST_RESULTS
    import os
    ins = {k: np.ascontiguousarray(np.asarray(v, dtype=np.float32))
           for k, v in inputs.items()}
    if _NC_CACHE is None:
        _NC_CACHE = build_program()
    nc = _NC_CACHE

    import ml_dtypes
    bf = ml_dtypes.bfloat16

    def col6(v):  # [768] -> [128, 6] matching "(ct p) -> p ct"
        return np.ascontiguousarray(v.reshape(6, 128).T)

    cvec = np.concatenate(
        [col6(ins["ln1_g"]), col6(ins["ln1_b"]), col6(ins["ln2_g"]),
         col6(ins["ln2_b"]), col6(ins["proj_b"]), col6(ins["fc2_b"]),
         np.ascontiguousarray(ins["fc1_b"].reshape(HT, 128).T)],
        axis=1).astype(np.float32)

    shared = {
        "wqkvT": np.ascontiguousarray(ins["qkv_w"].T.astype(bf)),
        "wprojT": np.ascontiguousarray(ins["proj_w"].T.astype(bf)),
        "wfc1T": np.ascontiguousarray(ins["fc1_w"].T.astype(bf)),
        "wfc2T": np.ascontiguousarray(ins["fc2_w"].T.astype(bf)),
        "cvec": np.ascontiguousarray(cvec),
    }
    in_maps = []
    for s in range(NCORES):
        b, half = s // 2, s % 2
        m = dict(shared)
        m["xb"] = np.ascontiguousarray(np.roll(ins["x"][b], -half * NO, axis=0))
        in_maps.append(m)

    trace = bool(int(os.environ.get("KBENCH_TRACE", "0")))
    LAST_RESULTS = run_bass_kernel_spmd(
        nc, in_maps, core_ids=list(range(NCORES)), trace=trace)
    out = np.empty((B, N, C), np.float32)
    for s in range(NCORES):
        b, half = s // 2, s % 2
        out[b, half * NO:(half + 1) * NO, :] = LAST_RESULTS.results[s]["outT"].T
    return out


# revision 40
# speedup vs baseline: 1.1153x; 1.0416x over previous
"""Trainium2 Bass kernel for a dense transformer block (B=4, N=2048, C=768,
H=12, D=64, HID=3072), sharded over 8 NeuronCores.

Sharding: token-split, no collectives. Core s handles batch b = s//2,
sequence half = s%2 (1024 tokens). Each core receives its batch element's
full 2048-token x (rolled so its own tokens are rows 0..1023), computes
K/V over all 2048 tokens (redundantly with its pair core), and produces
the output for its own 1024 tokens. Host gathers/transposes.

v3 schedule (trace-driven rewrite of v2):
- era1 interleaves per 512-token group: x DMA -> raw-x^T transposes (PE
  work with no LN dependency) -> LN1 -> h^T transposes -> V -> K/Q
  chunks, so PE/Scalar/DVE overlap from t~2us. Consts land in ONE
  batched DMA (cvec).
- attention is ScalarE(exp)-bound; PE fillers are fine-grained QUANTA
  (1 matmul each) pulled one-per-step inside the S->exp->AV pipeline,
  with per-head-pair deadlines (kq ft3-5 feed chunk0's own later head
  pairs; proj/LN2/h2/fc1-staging of chunk0 run during chunk1).
- softmax denominators: batched reciprocal split in two halves per
  chunk so oT normalization starts 3 head-pairs early and proj quanta
  are ready at chunk1 start / tail start.
- all rstd use Exp(-0.5*Ln(var+eps)) -- same activation table set as
  the softmax exp (natural_log_exp_and_others); Gelu is the only other
  set, so 2 ACT table loads total instead of 12.
- weights prefetch: wproj+wfc1(h0) at chunk0 start, wfc1(h1) at chunk1
  start, wfc2(h1) at tail start -- tail never waits on DMA.
- tail order: proj(1) -> ln2(1)mm -> fc2(0,0) -> fc1(0,1) -> fc2(0,1)
  -> fc1(1,0) -> fc2(1,0) -> fc1(1,1) -> fc2(1,1), gelu fused from
  PSUM, so the LN2(1) serial chain hides under fc2/fc1 matmuls.
"""

from contextlib import ExitStack

import numpy as np

import concourse.bass as bass
import concourse.mybir as mybir
import concourse.tile as tile
from concourse import bacc
from concourse.bass_utils import run_bass_kernel_spmd
from concourse.masks import make_identity

F32 = mybir.dt.float32
BF16 = mybir.dt.bfloat16
AF = mybir.ActivationFunctionType
ALU = mybir.AluOpType

B, N, C = 4, 2048, 768
H, D = 12, 64
HID = 3072
EPS = 1e-5
NCORES = 8
NO = 1024  # tokens owned per core
NKV = 2048  # key/value tokens per core
CT = C // 128  # 6 feature tiles
HT = HID // 128  # 24 hidden tiles
HQ = HT // 2  # 12 hidden tiles per half
KT = NKV // 128  # 16 kv token tiles
QCH = NO // 512  # 2 query chunks of 512
ISCALE = 1.0 / np.sqrt(D)
SKEW = 2  # S->AV software-pipeline depth (in nt steps)
NCV = 36 + HT  # cvec columns: g1,b1,g2,b2,pb,f2b (6 ea) + f1b (24)

LAST_RESULTS = None
_NC_CACHE = None


def build_program(repeats=1):
    nc = bacc.Bacc(trn_type="TRN2", target_bir_lowering=False, num_devices=NCORES)

    xb = nc.dram_tensor("xb", [NKV, C], F32, kind="ExternalInput").ap()
    wqkvT = nc.dram_tensor("wqkvT", [C, 3 * C], BF16, kind="ExternalInput").ap()
    wprojT = nc.dram_tensor("wprojT", [C, C], BF16, kind="ExternalInput").ap()
    wfc1T = nc.dram_tensor("wfc1T", [C, HID], BF16, kind="ExternalInput").ap()
    wfc2T = nc.dram_tensor("wfc2T", [HID, C], BF16, kind="ExternalInput").ap()
    cvec = nc.dram_tensor("cvec", [128, NCV], F32, kind="ExternalInput").ap()
    outT = nc.dram_tensor("outT", [C, NO], F32, kind="ExternalOutput").ap()

    with tile.TileContext(nc) as tc:
        for _ in range(repeats):
            emit(nc, tc, xb, wqkvT, wprojT, wfc1T, wfc2T, cvec, outT)
    nc.compile()
    return nc


def emit(nc, tc, xb, wqkvT, wprojT, wfc1T, wfc2T, cvec, outT):
    dma = nc.sync.dma_start
    qs = (slice(0, 64), slice(64, 128))

    with ExitStack() as _st:
        def pool(**kw):
            return _st.enter_context(tc.tile_pool(**kw))

        consts = pool(name="consts", bufs=1)
        x2T_pool = pool(name="x2T", bufs=1)
        xoT_pool = pool(name="xoT", bufs=1)

        # ---- constants: one batched DMA for every small vector ----
        cv = consts.tile([128, NCV], F32, tag="cv")
        dma(out=cv, in_=cvec)
        g1_s = cv[:, 0:6]
        b1_s = cv[:, 6:12]
        g2_s = cv[:, 12:18]
        b2_s = cv[:, 18:24]
        pb_s = cv[:, 24:30]
        f2b_s = cv[:, 30:36]
        f1b_s = cv[:, 36:36 + HT]

        ident = consts.tile([128, 128], F32, tag="ident")
        make_identity(nc, ident)
        ident_bf = consts.tile([128, 128], BF16, tag="ident_bf")
        make_identity(nc, ident_bf)
        ones_f32 = consts.tile([128, 1], F32, tag="ones_f32")
        nc.vector.memset(ones_f32, 1.0)
        ones_cb = consts.tile([128, 1], BF16, tag="ones_cb")
        nc.scalar.activation(out=ones_cb, in_=ones_f32, func=AF.Copy)
        eps_t = consts.tile([128, 1], F32, tag="eps")
        nc.vector.memset(eps_t, EPS)

        # residual streams (bf16 to fit SBUF; rel-err budget is 2e-2).
        # xoT holds the host-transposed raw x^T for the OWN tokens -- it
        # doubles as the residual stream (no PE transposes anywhere).
        x2T = [x2T_pool.tile([128, NO], BF16, tag=f"x2T{ct}", name=f"x2T{ct}")
               for ct in range(CT)]
        xoT = [xoT_pool.tile([128, NO], BF16, tag=f"xoT{ct}", name=f"xoT{ct}")
               for ct in range(CT)]

        # ---- PSUM pools: sps 4 banks, ops 2, mps 2 (8 total) ----
        sps = pool(name="s_psum", bufs=2, space="PSUM")
        ops = pool(name="o_psum", bufs=2, space="PSUM")
        mps = pool(name="m_psum", bufs=2, space="PSUM")

        oU_pool = pool(name="oU", bufs=12)
        oT_pool = pool(name="oT", bufs=6)
        den_pool = pool(name="den", bufs=1)
        asm = pool(name="asm", bufs=2)

        _stk = ExitStack()  # attention working set: freed at tail

        def kpool(**kw):
            return _stk.enter_context(tc.tile_pool(**kw))

        qT_pool = kpool(name="qT", bufs=1)
        kT_pool = kpool(name="kT", bufs=1)
        vA_pool = kpool(name="vA", bufs=1)
        p_sb = kpool(name="p_sb", bufs=4)
        qT = [qT_pool.tile([128, NO], BF16, tag=f"qT{ct}", name=f"qT{ct}")
              for ct in range(CT)]
        kT = [kT_pool.tile([128, NKV], BF16, tag=f"kT{ct}", name=f"kT{ct}")
              for ct in range(CT)]
        vA = [vA_pool.tile([128, H, D + 1], BF16, tag=f"vA{nt}",
                           name=f"vA{nt}") for nt in range(KT)]

        dent = [None, None]
        oT = [[None] * CT for _ in range(QCH)]
        oU_all = [[None] * H for _ in range(QCH)]
        ln2v = [None, None]
        h2c = [[None] * CT for _ in range(QCH)]

        _hstk = ExitStack()  # hkvT/wq: freed at chunk0|chunk1 boundary
        hkvT_pool = _hstk.enter_context(tc.tile_pool(name="hkvT", bufs=1))
        wq_pool = _hstk.enter_context(tc.tile_pool(name="wqkv", bufs=1))
        hkvT = [hkvT_pool.tile([128, NKV], BF16, tag=f"hkvT{ct}",
                               name=f"hkvT{ct}") for ct in range(CT)]
        wq = wq_pool.tile([128, CT, 3 * C], BF16, tag="wqkv")
        # wq rides the scalar-engine DMA path in K,V,Q column order so the
        # sync queue stays clear for x-tile prefetch and each slice lands
        # just before its first consumer in group 0.
        _wq_src = wqkvT.rearrange("(ct p) f -> p ct f", p=128)
        nc.scalar.dma_start(out=wq[:, :, C:2 * C], in_=_wq_src[:, :, C:2 * C])
        nc.scalar.dma_start(out=wq[:, :, 2 * C:3 * C],
                            in_=_wq_src[:, :, 2 * C:3 * C])
        nc.scalar.dma_start(out=wq[:, :, 0:C], in_=_wq_src[:, :, 0:C])

        # =================== era1: per-group interleave ===================
        def v_tile(nt):
            """V for one kv token tile -> vA[nt] (sps psum, 3 banks used)."""
            psAB = sps.tile([128, 1024], F32, tag="ps", name=f"psAB{nt}")
            for ct in range(CT):
                hk = hkvT[ct][:, nt * 128:(nt + 1) * 128]
                nc.tensor.matmul(psAB[:, 0:512], hk, wq[:, ct, 2 * C:2 * C + 512],
                                 start=(ct == 0), stop=(ct == CT - 1))
                nc.tensor.matmul(psAB[:, 512:768], hk,
                                 wq[:, ct, 2 * C + 512:3 * C],
                                 start=(ct == 0), stop=(ct == CT - 1))
            nc.vector.tensor_copy(
                out=vA[nt][:, 0:8, 0:D],
                in_=psAB[:, 0:512].rearrange("p (h d) -> p h d", d=D))
            nc.vector.tensor_copy(
                out=vA[nt][:, 8:12, 0:D],
                in_=psAB[:, 512:768].rearrange("p (h d) -> p h d", d=D))
            nc.vector.memset(vA[nt][:, :, D:D + 1], 1.0)

        def k_chunk(ft, chk):
            ps = mps.tile([128, 512], F32, tag="mp", name=f"kc{ft}_{chk}")
            for ct in range(CT):
                nc.tensor.matmul(
                    ps, wq[:, ct, C + ft * 128:C + (ft + 1) * 128],
                    hkvT[ct][:, chk * 512:(chk + 1) * 512],
                    start=(ct == 0), stop=(ct == CT - 1))
                if ct < CT - 1:
                    yield
            nc.vector.tensor_copy(
                out=kT[ft][:, chk * 512:(chk + 1) * 512], in_=ps)
            yield

        def q_chunk(ft, chk):
            ps = mps.tile([128, 512], F32, tag="mp", name=f"qc{ft}_{chk}")
            for ct in range(CT):
                nc.tensor.matmul(
                    ps, wq[:, ct, ft * 128:(ft + 1) * 128],
                    hkvT[ct][:, chk * 512:(chk + 1) * 512],
                    start=(ct == 0), stop=(ct == CT - 1))
                if ct < CT - 1:
                    yield
            nc.vector.tensor_copy(
                out=qT[ft][:, chk * 512:(chk + 1) * 512], in_=ps)
            yield

        def run_gen(g):
            for _ in g:
                pass

        with (
            tc.tile_pool(name="xw", bufs=2) as xpool,
            tc.tile_pool(name="ln1_stat", bufs=6) as lstat,
        ):
            for g in range(KT // 4):  # groups of 4 token tiles (512 tokens)
                xts, xcs = [], []
                for j in range(4):
                    nt = 4 * g + j
                    xt = xpool.tile([128, C], F32, tag=f"xt{j}", name=f"xt{j}")
                    dma(out=xt, in_=xb[nt * 128:(nt + 1) * 128, :])
                    st = lstat.tile([128, 3, 6], F32, tag="st")
                    xg = xt.rearrange("p (s d) -> p s d", s=3)
                    for s in range(3):
                        nc.vector.bn_stats(out=st[:, s], in_=xg[:, s])
                    mv = lstat.tile([128, 2], F32, tag="mv")
                    nc.vector.bn_aggr(out=mv, in_=st)
                    rstd = lstat.tile([128, 1], F32, tag="rstd")
                    nc.scalar.activation(out=rstd, in_=mv[:, 1:2],
                                         func=AF.Sqrt, bias=eps_t, scale=1.0)
                    nc.vector.reciprocal(out=rstd, in_=rstd)
                    nmr = lstat.tile([128, 1], F32, tag="nmr")
                    nc.vector.tensor_scalar(out=nmr, in0=mv[:, 0:1],
                                            scalar1=-1.0, scalar2=rstd,
                                            op0=ALU.mult, op1=ALU.mult)
                    xc = xpool.tile([128, C], BF16, tag=f"xc{j}",
                                    name=f"xc{j}", bufs=1)
                    nc.scalar.activation(out=xc, in_=xt, func=AF.Identity,
                                         scale=rstd, bias=nmr)
                    xts.append(xt)
                    xcs.append(xc)
                # interleave raw-x^T (DVE drain) and h^T (ACT drain)
                # transposes so the two drain engines run concurrently.
                for ct in range(CT):
                    if g < NO // 512:
                        ps32 = mps.tile([128, 512], F32, tag="mp",
                                        name="ps32")
                        for j in range(4):
                            nc.tensor.transpose(
                                ps32[:, j * 128:(j + 1) * 128],
                                xts[j][:, ct * 128:(ct + 1) * 128], ident)
                        nc.vector.tensor_copy(
                            out=xoT[ct][:, g * 512:(g + 1) * 512], in_=ps32)
                    ps = mps.tile([128, 512], BF16, tag="mp", name="pst")
                    for j in range(4):
                        nc.tensor.transpose(
                            ps[:, j * 128:(j + 1) * 128],
                            xcs[j][:, ct * 128:(ct + 1) * 128], ident_bf)
                    nc.scalar.activation(
                        out=hkvT[ct][:, g * 512:(g + 1) * 512],
                        in_=ps, func=AF.Identity,
                        scale=g1_s[:, ct:ct + 1], bias=b1_s[:, ct:ct + 1])
                for ft in range(3):  # ft0-2 K here; ft3-5 are attn fillers
                    run_gen(k_chunk(ft, g))
                for j in range(4):
                    v_tile(4 * g + j)
                if g < QCH:
                    for ft in range(3):
                        run_gen(q_chunk(ft, g))

        # =================== attention machinery ===================
        def ot_norm_piece(ch, hp, rec, base):
            """Normalize one head pair's output into oT[ch][hp]."""
            t = oT_pool.tile([128, 512], BF16, tag="oT",
                             name=f"oT{ch}_{hp}")
            for i in range(2):
                r = 2 * hp + i
                rb = asm.tile([1, 512], BF16, tag="rb")
                dma(out=rb, in_=rec[r - base:r - base + 1, :])
                vb = asm.tile([D, 512], BF16, tag="vb")
                nc.gpsimd.partition_broadcast(vb, rb, channels=D)
                nc.vector.tensor_mul(t[qs[i], :], oU_all[ch][r], vb)
            oT[ch][hp] = t

        def rec_quarter(ch, q):
            """Batched reciprocal of 4 denominator rows (heads 4q..4q+3),
            then normalize head pairs 2q and 2q+1 -- runs as soon as each
            third of a chunk's denominators lands so oT is ready early."""
            rec = dent[ch][q]
            with nc.allow_low_precision(reason="softmax denom in bf16"):
                nc.vector.reciprocal(out=rec, in_=rec)
            for hp in (2 * q, 2 * q + 1):
                ot_norm_piece(ch, hp, rec, 4 * q)

        class Feed:
            """Deadline-ordered filler quanta. pull() emits one quantum;
            flush(hp) force-emits every generator due before head pair hp
            (PE is in-order, so a quantum consumed by hp's matmuls must be
            emitted before them)."""

            def __init__(self):
                self.items = []

            def add(self, deadline, gen):
                self.items.append([deadline, gen])

            def pull(self):
                for it in self.items:
                    if it[1] is not None:
                        try:
                            next(it[1])
                            return True
                        except StopIteration:
                            it[1] = None
                return False

            def flush(self, hp):
                for it in self.items:
                    if it[1] is not None and it[0] <= hp:
                        for _ in it[1]:
                            pass
                        it[1] = None

            def flush_all(self):
                self.flush(10 ** 9)

        def attention_all(feed_of, boundary_hook):
            """Both chunks as ONE continuously-skewed (ch, hp, nt) stream:
            the next head pair's S matmuls interleave with the previous
            pair's AV tail, so the exp stream never pauses at boundaries.
            feed_of(ch) -> Feed; boundary_hook(ch) fires before chunk ch's
            first S (pool swaps / weight prefetch / feed construction)."""
            seq = [(ch, hp, nt) for ch in range(QCH)
                   for hp in range(CT) for nt in range(KT)]
            po_cur = {}
            pts = {}
            feed = None
            for idx in range(len(seq) + SKEW):
                if idx < len(seq):
                    ch, hp, nt = seq[idx]
                    if nt == 0:
                        if hp == 0:
                            if feed is not None:
                                feed.flush_all()
                            boundary_hook(ch)
                            feed = feed_of(ch)
                            dent[ch] = [
                                den_pool.tile([4, 512], BF16, tag=f"dent{h}",
                                              name=f"dent{ch}_{h}")
                                for h in range(3)]
                        feed.flush(hp)
                    qch = slice(ch * 512, (ch + 1) * 512)
                    ps2 = sps.tile([128, 1024], F32, tag="ps")
                    for i in range(2):
                        nc.tensor.matmul(
                            ps2[:, i * 512:(i + 1) * 512],
                            kT[hp][qs[i], nt * 128:(nt + 1) * 128],
                            qT[hp][qs[i], qch],
                            start=True, stop=True,
                            tile_position=(64 * i, 0))
                    pt2 = p_sb.tile([128, 1024], BF16, tag="pt")
                    nc.scalar.activation(out=pt2, in_=ps2,
                                         func=AF.Exp, scale=ISCALE)
                    pts[(ch, hp, nt)] = pt2
                if idx >= SKEW:
                    ch_a, hp_a, m = seq[idx - SKEW]
                    pt2 = pts.pop((ch_a, hp_a, m))
                    if m == 0:
                        po_cur[(ch_a, hp_a)] = [
                            ops.tile([D + 1, 512], F32, tag="po",
                                     name=f"po{ch_a}_{hp_a}_{i}")
                            for i in range(2)]
                    po = po_cur[(ch_a, hp_a)]
                    for i in range(2):
                        nc.tensor.matmul(
                            po[i], vA[m][:, 2 * hp_a + i, :],
                            pt2[:, i * 512:(i + 1) * 512],
                            start=(m == 0), stop=(m == KT - 1),
                            skip_group_check=True)
                    if m == KT - 1:
                        # drain: unnormalized O^T halves + denominator row
                        for i in range(2):
                            r = 2 * hp_a + i
                            oU = oU_pool.tile([D, 512], BF16, tag="oU",
                                              name=f"oU{ch_a}_{r}")
                            nc.vector.tensor_copy(out=oU, in_=po[i][0:D, :])
                            oU_all[ch_a][r] = oU
                            dt = asm.tile([1, 512], BF16, tag="dtmp")
                            nc.vector.tensor_copy(out=dt, in_=po[i][D:D + 1, :])
                            dma(out=dent[ch_a][r // 4]
                                [(r % 4):(r % 4) + 1, :], in_=dt)
                        del po_cur[(ch_a, hp_a)]
                        if hp_a % 2 == 1:
                            rec_quarter(ch_a, hp_a // 2)
                # filler pulls: ~1.5/step keeps PE full without starving
                # the exp stream, and drains inventory before chunk end
                feed.pull()
                if idx % 2 == 0:
                    feed.pull()

        # ------------- proj / LN2 / MLP building blocks -------------
        def proj_quanta(ch, wp):
            """proj + x2 residual for chunk ch. f0/f1 start on the first
            five oT tiles so quanta are ready before oT[5] lands."""
            cs = slice(ch * 512, (ch + 1) * 512)
            ps01 = []
            for f in range(2):
                ps = mps.tile([128, 512], F32, tag="mp", name=f"pj{f}")
                ps01.append(ps)
                for ct in range(CT - 1):
                    nc.tensor.matmul(
                        ps, wp[:, ct, f * 128:(f + 1) * 128], oT[ch][ct],
                        start=(ct == 0), stop=False)
                    yield
            for f in range(2):
                nc.tensor.matmul(
                    ps01[f], wp[:, CT - 1, f * 128:(f + 1) * 128],
                    oT[ch][CT - 1], start=False, stop=True)
                nc.vector.scalar_tensor_tensor(
                    out=x2T[f][:, cs], in0=ps01[f],
                    scalar=pb_s[:, f:f + 1], in1=xoT[f][:, cs],
                    op0=ALU.add, op1=ALU.add)
                yield
            for f in range(2, CT):
                ps = mps.tile([128, 512], F32, tag="mp", name=f"pj{f}")
                for ct in range(CT):
                    nc.tensor.matmul(
                        ps, wp[:, ct, f * 128:(f + 1) * 128], oT[ch][ct],
                        start=(ct == 0), stop=(ct == CT - 1))
                    if ct < CT - 1:
                        yield
                nc.vector.scalar_tensor_tensor(
                    out=x2T[f][:, cs], in0=ps,
                    scalar=pb_s[:, f:f + 1], in1=xoT[f][:, cs],
                    op0=ALU.add, op1=ALU.add)
                yield

        def ln2_quanta(ch, lw):
            cs = slice(ch * 512, (ch + 1) * 512)
            psum = mps.tile([1, 512], F32, tag="mp", name="psum")
            pssq = mps.tile([1, 512], F32, tag="mp", name="pssq")
            for ct in range(CT):
                sq = lw.tile([128, 512], BF16, tag="sq", bufs=1)
                nc.vector.tensor_mul(sq, x2T[ct][:, cs], x2T[ct][:, cs])
                nc.tensor.matmul(psum, ones_cb, x2T[ct][:, cs],
                                 start=(ct == 0), stop=(ct == CT - 1),
                                 skip_group_check=True)
                nc.tensor.matmul(pssq, ones_cb, sq,
                                 start=(ct == 0), stop=(ct == CT - 1),
                                 skip_group_check=True)
                yield
            # serial stats chain (sqrt lands at a head-pair boundary so the
            # one table round-trip hides under S/AV matmuls)
            mu = lw.tile([1, 512], F32, tag="mu", name=f"mu{ch}", bufs=1)
            nc.vector.tensor_scalar_mul(mu, psum, 1.0 / C)
            mu2 = lw.tile([1, 512], F32, tag="mu2", bufs=1)
            nc.vector.tensor_mul(mu2, mu, mu)
            var = lw.tile([1, 512], F32, tag="var", bufs=1)
            nc.vector.scalar_tensor_tensor(
                out=var, in0=pssq, scalar=1.0 / C, in1=mu2,
                op0=ALU.mult, op1=ALU.subtract)
            std = lw.tile([1, 512], F32, tag="mu2", name="std", bufs=1)
            nc.scalar.activation(out=std, in_=var, func=AF.Sqrt,
                                 bias=eps_t[0:1], scale=1.0)
            rstdf = lw.tile([1, 512], F32, tag="rstdf", bufs=1)
            nc.vector.reciprocal(out=rstdf, in_=std)
            nmr = lw.tile([1, 512], BF16, tag=f"nmr2_{ch}", name="nmr2",
                          bufs=1)
            nc.vector.scalar_tensor_tensor(
                out=nmr, in0=mu, scalar=-1.0, in1=rstdf,
                op0=ALU.mult, op1=ALU.mult)
            rstd = lw.tile([1, 512], BF16, tag=f"rstd2_{ch}", name="rstd2",
                           bufs=1)
            nc.vector.tensor_copy(out=rstd, in_=rstdf)
            ln2v[ch] = (rstd, nmr)
            yield

        def h2_quanta(ch, lw, h2_pool, bc_pool):
            """h2 = ((x2T*bc_r + bc_m)*g2 + b2): broadcast on gpsimd,
            affine on DVE (keeps ScalarE free for exps)."""
            cs = slice(ch * 512, (ch + 1) * 512)
            rstd, nmr = ln2v[ch]
            bc_r = bc_pool.tile([128, 512], BF16, tag="bc_r")
            nc.gpsimd.partition_broadcast(bc_r, rstd, channels=128)
            bc_m = bc_pool.tile([128, 512], BF16, tag="bc_m")
            nc.gpsimd.partition_broadcast(bc_m, nmr, channels=128)
            yield
            for ct in range(CT):
                t = lw.tile([128, 512], BF16, tag="h2tmp")
                nc.vector.tensor_mul(t, x2T[ct][:, cs], bc_r)
                nc.vector.tensor_add(t, t, bc_m)
                h2 = h2_pool.tile([128, 512], BF16,
                                  tag=f"h2_{ct}_{ch}", name=f"h2_{ct}_{ch}")
                nc.vector.tensor_scalar(
                    out=h2, in0=t, scalar1=g2_s[:, ct:ct + 1],
                    scalar2=b2_s[:, ct:ct + 1],
                    op0=ALU.mult, op1=ALU.add)
                h2c[ch][ct] = h2
                yield

        # fc1 matmuls only, staging pre-gelu h1 in bf16 (gelu deferred so
        # ScalarE stays on the exp table set during attention)
        def fc1_stage_quanta(ch, hb, w1t, gbuf, h1s):
            for ht in range(HQ):
                ps = mps.tile([128, 512], F32, tag="mp", name="f1")
                for ct in range(CT):
                    nc.tensor.matmul(
                        ps, w1t[:, ct, ht * 128:(ht + 1) * 128],
                        h2c[ch][ct],
                        start=(ct == 0), stop=(ct == CT - 1))
                    if ct < CT - 1:
                        yield
                h1 = gbuf.tile([128, 512], BF16, tag="gb",
                               name=f"h1_{ch}_{hb}_{ht}")
                nc.vector.tensor_copy(out=h1, in_=ps)
                h1s[ht] = h1
                yield

        def gelu_piece(ch, hb, gbuf, h1s, gs):
            for ht in range(HQ):
                g = gbuf.tile([128, 512], BF16, tag="gb",
                              name=f"g_{ch}_{hb}_{ht}")
                hti = hb * HQ + ht
                nc.scalar.activation(out=g, in_=h1s[ht], func=AF.Gelu,
                                     bias=f1b_s[:, hti:hti + 1], scale=1.0)
                gs[ht] = g

        # fc1 with gelu fused right off PSUM (post-attention phases)
        def fc1_full(ch, hb, w1t, gbuf, gs):
            for ht in range(HQ):
                ps = mps.tile([128, 512], F32, tag="mp", name="f1")
                for ct in range(CT):
                    nc.tensor.matmul(
                        ps, w1t[:, ct, ht * 128:(ht + 1) * 128],
                        h2c[ch][ct],
                        start=(ct == 0), stop=(ct == CT - 1))
                g = gbuf.tile([128, 512], BF16, tag="gb",
                              name=f"g_{ch}_{hb}_{ht}")
                hti = hb * HQ + ht
                nc.scalar.activation(out=g, in_=ps, func=AF.Gelu,
                                     bias=f1b_s[:, hti:hti + 1], scale=1.0)
                gs[ht] = g

        acc = {}

        def fc2_piece(ch, hb, w2t, gs, accp, osb):
            cs = slice(ch * 512, (ch + 1) * 512)
            for ft in range(CT):
                ps = mps.tile([128, 512], F32, tag="mp", name="f2")
                for ht in range(HQ):
                    nc.tensor.matmul(
                        ps, w2t[:, ht, ft * 128:(ft + 1) * 128], gs[ht],
                        start=(ht == 0), stop=(ht == HQ - 1))
                if hb == 0:
                    a = accp.tile([128, 512], BF16, tag=f"acc{ft}_{ch}",
                                  name=f"acc{ft}_{ch}")
                    acc[(ft, ch)] = a
                    nc.vector.scalar_tensor_tensor(
                        out=a, in0=ps,
                        scalar=f2b_s[:, ft:ft + 1],
                        in1=x2T[ft][:, cs],
                        op0=ALU.add, op1=ALU.add)
                else:
                    ot = osb.tile([128, 512], F32, tag="ot")
                    nc.vector.tensor_add(ot, ps, acc[(ft, ch)])
                    dma(out=outT[ft * 128:(ft + 1) * 128, cs], in_=ot)

        # =================== attention (both chunks, flat) ===================
        env = {}
        h1s0 = [None] * HQ  # staged pre-gelu fc1 outputs (ch0, hb0)
        HH = HID // 2

        def boundary(ch):
            if ch == 1:
                # hkvT/wq freed -> SBUF headroom for the MLP working set;
                # wproj + wfc1(h0) DMAs ride under chunk 1 (their quanta
                # start ~25us in).
                _hstk.close()
                env["wp_pool"] = pool(name="wproj", bufs=1, side="right")
                env["w1_pool"] = pool(name="wfc1", bufs=1, side="right")
                env["gbuf"] = pool(name="gbuf", bufs=13, side="right")
                env["h2a_pool"] = pool(name="h2a_sb", bufs=1, side="right")
                env["acc_pool"] = pool(name="acc_sb", bufs=1, side="right")
                env["bc_pool"] = pool(name="bc_sb", bufs=1, side="right")
                env["lw"] = pool(name="mlp_work", bufs=1, side="right")
                wp = env["wp_pool"].tile([128, CT, C], BF16, tag="wproj")
                dma(out=wp, in_=wprojT.rearrange("(ct p) f -> p ct f", p=128))
                env["wp"] = wp
                w1t0 = env["w1_pool"].tile([128, CT, HH], BF16, tag="wfc1",
                                           name="w1h0")
                dma(out=w1t0,
                    in_=wfc1T.rearrange("(ct p) f -> p ct f", p=128)
                    [:, :, 0:HH])
                env["w1t0"] = w1t0

        def mk_feed(ch):
            f = Feed()
            if ch == 0:
                for ft in (3, 4, 5):
                    for chk in range(4):
                        f.add(ft, k_chunk(ft, chk))
                    for chk in range(QCH):
                        f.add(ft, q_chunk(ft, chk))
            else:
                f.add(99, proj_quanta(0, env["wp"]))
                f.add(99, ln2_quanta(0, env["lw"]))
                f.add(99, h2_quanta(0, env["lw"], env["h2a_pool"],
                                    env["bc_pool"]))
                f.add(99, fc1_stage_quanta(0, 0, env["w1t0"], env["gbuf"],
                                           h1s0))
            return f

        attention_all(mk_feed, boundary)
        _stk.close()  # free qT/kT/vA/pt pools for the MLP tail
        wp, w1t0 = env["wp"], env["w1t0"]
        gbuf, lw = env["gbuf"], env["lw"]

        # =================== MLP tail ===================
        w2_pool = pool(name="wfc2", bufs=1, side="right")
        h2b_pool = pool(name="h2b_sb", bufs=1, side="right")
        accb_pool = pool(name="accb_sb", bufs=1, side="right")
        osb = pool(name="out_sb", bufs=2, side="right")
        w2t0 = w2_pool.tile([128, HQ, C], BF16, tag="wfc2", name="w2h0")
        dma(out=w2t0,
            in_=wfc2T.rearrange("(ht p) f -> p ht f", p=128)[:, 0:HQ, :])
        w1t1 = env["w1_pool"].tile([128, CT, HH], BF16, tag="wfc1b",
                                   name="w1h1")
        dma(out=w1t1,
            in_=wfc1T.rearrange("(ct p) f -> p ct f", p=128)[:, :, HH:HID])
        w2t1 = w2_pool.tile([128, HQ, C], BF16, tag="wfc2b", name="w2h1")
        dma(out=w2t1,
            in_=wfc2T.rearrange("(ht p) f -> p ht f", p=128)[:, HQ:HT, :])

        # gelu00 + fc2(0,0) first: fc2's f0 matmuls trail the gelu stream
        # (only tail work with no dependence on chunk1's oT), then proj(1)
        # once oT(1) lands, then the rest at full PE rate.
        g00 = [None] * HQ
        gelu_piece(0, 0, gbuf, h1s0, g00)
        fc2_piece(0, 0, w2t0, g00, env["acc_pool"], None)
        run_gen(proj_quanta(1, wp))
        run_gen(ln2_quanta(1, lw))
        g01 = [None] * HQ
        fc1_full(0, 1, w1t1, gbuf, g01)
        run_gen(h2_quanta(1, lw, h2b_pool, env["bc_pool"]))
        fc2_piece(0, 1, w2t1, g01, None, osb)
        g10 = [None] * HQ
        fc1_full(1, 0, w1t0, gbuf, g10)
        fc2_piece(1, 0, w2t0, g10, accb_pool, None)
        g11 = [None] * HQ
        fc1_full(1, 1, w1t1, gbuf, g11)
        fc2_piece(1, 1, w2t1, g11, None, osb)


def kernel(**inputs):
    global _NC_CACHE, LAST_RESULTS
    import os
    ins = {k: np.ascontiguousarray(np.asarray(v, dtype=np.float32))
           for k, v in inputs.items()}
    if _NC_CACHE is None:
        _NC_CACHE = build_program()
    nc = _NC_CACHE

    import ml_dtypes
    bf = ml_dtypes.bfloat16

    def col6(v):  # [768] -> [128, 6] matching "(ct p) -> p ct"
        return np.ascontiguousarray(v.reshape(6, 128).T)

    cvec = np.concatenate(
        [col6(ins["ln1_g"]), col6(ins["ln1_b"]), col6(ins["ln2_g"]),
         col6(ins["ln2_b"]), col6(ins["proj_b"]), col6(ins["fc2_b"]),
         np.ascontiguousarray(ins["fc1_b"].reshape(HT, 128).T)],
        axis=1).astype(np.float32)

    shared = {
        "wqkvT": np.ascontiguousarray(ins["qkv_w"].T.astype(bf)),
        "wprojT": np.ascontiguousarray(ins["proj_w"].T.astype(bf)),
        "wfc1T": np.ascontiguousarray(ins["fc1_w"].T.astype(bf)),
        "wfc2T": np.ascontiguousarray(ins["fc2_w"].T.astype(bf)),
        "cvec": np.ascontiguousarray(cvec),
    }
    in_maps = []
    for s in range(NCORES):
        b, half = s // 2, s % 2
        m = dict(shared)
        m["xb"] = np.ascontiguousarray(np.roll(ins["x"][b], -half * NO, axis=0))
        in_maps.append(m)

    trace = bool(int(os.environ.get("KBENCH_TRACE", "0")))
    LAST_RESULTS = run_bass_kernel_spmd(
        nc, in_maps, core_ids=list(range(NCORES)), trace=trace)
    out = np.empty((B, N, C), np.float32)
    for s in range(NCORES):
        b, half = s // 2, s % 2
        out[b, half * NO:(half + 1) * NO, :] = LAST_RESULTS.results[s]["outT"].T
    return out


# revision 41
# speedup vs baseline: 1.1532x; 1.0340x over previous
"""Trainium2 Bass kernel for a dense transformer block (B=4, N=2048, C=768,
H=12, D=64, HID=3072), sharded over 8 NeuronCores.

Sharding: token-split, no collectives. Core s handles batch b = s//2,
sequence half = s%2 (1024 tokens). Each core receives its batch element's
full 2048-token x (rolled so its own tokens are rows 0..1023), computes
K/V over all 2048 tokens (redundantly with its pair core), and produces
the output for its own 1024 tokens. Host gathers/transposes.

v3 schedule (trace-driven rewrite of v2):
- era1 interleaves per 512-token group: x DMA -> raw-x^T transposes (PE
  work with no LN dependency) -> LN1 -> h^T transposes -> V -> K/Q
  chunks, so PE/Scalar/DVE overlap from t~2us. Consts land in ONE
  batched DMA (cvec).
- attention is ScalarE(exp)-bound; PE fillers are fine-grained QUANTA
  (1 matmul each) pulled one-per-step inside the S->exp->AV pipeline,
  with per-head-pair deadlines (kq ft3-5 feed chunk0's own later head
  pairs; proj/LN2/h2/fc1-staging of chunk0 run during chunk1).
- softmax denominators: batched reciprocal split in two halves per
  chunk so oT normalization starts 3 head-pairs early and proj quanta
  are ready at chunk1 start / tail start.
- all rstd use Exp(-0.5*Ln(var+eps)) -- same activation table set as
  the softmax exp (natural_log_exp_and_others); Gelu is the only other
  set, so 2 ACT table loads total instead of 12.
- weights prefetch: wproj+wfc1(h0) at chunk0 start, wfc1(h1) at chunk1
  start, wfc2(h1) at tail start -- tail never waits on DMA.
- tail order: proj(1) -> ln2(1)mm -> fc2(0,0) -> fc1(0,1) -> fc2(0,1)
  -> fc1(1,0) -> fc2(1,0) -> fc1(1,1) -> fc2(1,1), gelu fused from
  PSUM, so the LN2(1) serial chain hides under fc2/fc1 matmuls.
"""

from contextlib import ExitStack

import numpy as np

import concourse.bass as bass
import concourse.mybir as mybir
import concourse.tile as tile
from concourse import bacc
from concourse.bass_utils import run_bass_kernel_spmd
from concourse.masks import make_identity

F32 = mybir.dt.float32
BF16 = mybir.dt.bfloat16
FP8 = mybir.dt.float8e4
WQS = 16.0  # host-side wqkv scale (fp8 range); S psums carry WQS^2
AF = mybir.ActivationFunctionType
ALU = mybir.AluOpType

B, N, C = 4, 2048, 768
H, D = 12, 64
HID = 3072
EPS = 1e-5
NCORES = 8
NO = 1024  # tokens owned per core
NKV = 2048  # key/value tokens per core
CT = C // 128  # 6 feature tiles
HT = HID // 128  # 24 hidden tiles
HQ = HT // 2  # 12 hidden tiles per half
KT = NKV // 128  # 16 kv token tiles
QCH = NO // 512  # 2 query chunks of 512
ISCALE = 1.0 / np.sqrt(D)
SKEW = 2  # S->AV software-pipeline depth (in nt steps)
NCV = 36 + HT  # cvec columns: g1,b1,g2,b2,pb,f2b (6 ea) + f1b (24)

LAST_RESULTS = None
_NC_CACHE = None


def build_program(repeats=1):
    nc = bacc.Bacc(trn_type="TRN2", target_bir_lowering=False, num_devices=NCORES)

    xb = nc.dram_tensor("xb", [NKV, C], F32, kind="ExternalInput").ap()
    wqkvT = nc.dram_tensor("wqkvT", [C, 3 * C], FP8, kind="ExternalInput").ap()
    wprojT = nc.dram_tensor("wprojT", [C, C], BF16, kind="ExternalInput").ap()
    wfc1T = nc.dram_tensor("wfc1T", [C, HID], BF16, kind="ExternalInput").ap()
    wfc2T = nc.dram_tensor("wfc2T", [HID, C], BF16, kind="ExternalInput").ap()
    cvec = nc.dram_tensor("cvec", [128, NCV], F32, kind="ExternalInput").ap()
    outT = nc.dram_tensor("outT", [C, NO], F32, kind="ExternalOutput").ap()

    with tile.TileContext(nc) as tc:
        for _ in range(repeats):
            emit(nc, tc, xb, wqkvT, wprojT, wfc1T, wfc2T, cvec, outT)
    nc.compile()
    return nc


def emit(nc, tc, xb, wqkvT, wprojT, wfc1T, wfc2T, cvec, outT):
    dma = nc.sync.dma_start
    qs = (slice(0, 64), slice(64, 128))

    with ExitStack() as _st:
        def pool(**kw):
            return _st.enter_context(tc.tile_pool(**kw))

        consts = pool(name="consts", bufs=1)
        x2T_pool = pool(name="x2T", bufs=1)
        xoT_pool = pool(name="xoT", bufs=1)

        # ---- constants: one batched DMA for every small vector ----
        cv = consts.tile([128, NCV], F32, tag="cv")
        dma(out=cv, in_=cvec)
        g1_s = cv[:, 0:6]
        b1_s = cv[:, 6:12]
        g2_s = cv[:, 12:18]
        b2_s = cv[:, 18:24]
        pb_s = cv[:, 24:30]
        f2b_s = cv[:, 30:36]
        f1b_s = cv[:, 36:36 + HT]

        ident = consts.tile([128, 128], F32, tag="ident")
        make_identity(nc, ident)
        ident_bf = consts.tile([128, 128], BF16, tag="ident_bf")
        make_identity(nc, ident_bf)
        ones_f32 = consts.tile([128, 1], F32, tag="ones_f32")
        nc.vector.memset(ones_f32, 1.0)
        ones_cb = consts.tile([128, 1], BF16, tag="ones_cb")
        nc.scalar.activation(out=ones_cb, in_=ones_f32, func=AF.Copy)
        eps_t = consts.tile([128, 1], F32, tag="eps")
        nc.vector.memset(eps_t, EPS)

        # residual streams (bf16 to fit SBUF; rel-err budget is 2e-2).
        # xoT holds the host-transposed raw x^T for the OWN tokens -- it
        # doubles as the residual stream (no PE transposes anywhere).
        x2T = [x2T_pool.tile([128, NO], BF16, tag=f"x2T{ct}", name=f"x2T{ct}")
               for ct in range(CT)]
        xoT = [xoT_pool.tile([128, NO], BF16, tag=f"xoT{ct}", name=f"xoT{ct}")
               for ct in range(CT)]

        # ---- PSUM pools: sps 4 banks, ops 2, mps 2 (8 total) ----
        sps = pool(name="s_psum", bufs=2, space="PSUM")
        ops = pool(name="o_psum", bufs=2, space="PSUM")
        mps = pool(name="m_psum", bufs=2, space="PSUM")

        oU_pool = pool(name="oU", bufs=12)
        oT_pool = pool(name="oT", bufs=6)
        den_pool = pool(name="den", bufs=1)
        asm = pool(name="asm", bufs=2)

        _stk = ExitStack()  # attention working set: freed at tail

        def kpool(**kw):
            return _stk.enter_context(tc.tile_pool(**kw))

        qT_pool = kpool(name="qT", bufs=1)
        kT_pool = kpool(name="kT", bufs=1)
        vA_pool = kpool(name="vA", bufs=1)
        p_sb = kpool(name="p_sb", bufs=4)
        qT = [qT_pool.tile([128, NO], BF16, tag=f"qT{ct}", name=f"qT{ct}")
              for ct in range(CT)]
        kT = [kT_pool.tile([128, NKV], BF16, tag=f"kT{ct}", name=f"kT{ct}")
              for ct in range(CT)]
        vA = [vA_pool.tile([128, H, D + 1], BF16, tag=f"vA{nt}",
                           name=f"vA{nt}") for nt in range(KT)]

        dent = [None, None]
        oT = [[None] * CT for _ in range(QCH)]
        oU_all = [[None] * H for _ in range(QCH)]
        ln2v = [None, None]
        h2c = [[None] * CT for _ in range(QCH)]

        _hstk = ExitStack()  # hkvT/wq: freed at chunk0|chunk1 boundary
        hkvT_pool = _hstk.enter_context(tc.tile_pool(name="hkvT", bufs=1))
        wq_pool = _hstk.enter_context(tc.tile_pool(name="wqkv", bufs=1))
        hkvA = hkvT_pool.tile([128, CT, NKV], FP8, tag="hkvA")
        hkvT = [hkvA[:, ct] for ct in range(CT)]
        wq = wq_pool.tile([128, CT, 3 * C], FP8, tag="wqkv")
        # wq rides the scalar-engine DMA path in K,V,Q column order so the
        # sync queue stays clear for x-tile prefetch and each slice lands
        # just before its first consumer in group 0.
        _wq_src = wqkvT.rearrange("(ct p) f -> p ct f", p=128)
        nc.scalar.dma_start(out=wq[:, :, C:2 * C], in_=_wq_src[:, :, C:2 * C])
        nc.scalar.dma_start(out=wq[:, :, 2 * C:3 * C],
                            in_=_wq_src[:, :, 2 * C:3 * C])
        nc.scalar.dma_start(out=wq[:, :, 0:C], in_=_wq_src[:, :, 0:C])

        # =================== era1: per-group interleave ===================
        def v_tile(nt):
            """V for one kv token tile -> vA[nt] (sps psum, 3 banks used)."""
            psAB = sps.tile([128, 1024], F32, tag="ps", name=f"psAB{nt}")
            for t in range(CT // 2):
                hk = hkvA[:, 2 * t:2 * t + 2, nt * 128:(nt + 1) * 128]
                nc.tensor.matmul(psAB[:, 0:512], hk,
                                 wq[:, 2 * t:2 * t + 2, 2 * C:2 * C + 512],
                                 start=(t == 0), stop=(t == CT // 2 - 1),
                                 perf_mode=mybir.MatmulPerfMode.DoubleRow)
                nc.tensor.matmul(psAB[:, 512:768], hk,
                                 wq[:, 2 * t:2 * t + 2, 2 * C + 512:3 * C],
                                 start=(t == 0), stop=(t == CT // 2 - 1),
                                 perf_mode=mybir.MatmulPerfMode.DoubleRow)
            nc.vector.tensor_scalar_mul(
                vA[nt][:, 0:8, 0:D],
                psAB[:, 0:512].rearrange("p (h d) -> p h d", d=D), 1.0 / WQS)
            nc.vector.tensor_scalar_mul(
                vA[nt][:, 8:12, 0:D],
                psAB[:, 512:768].rearrange("p (h d) -> p h d", d=D), 1.0 / WQS)
            nc.vector.memset(vA[nt][:, :, D:D + 1], 1.0)

        def k_chunk(ft, chk):
            ps = mps.tile([128, 512], F32, tag="mp", name=f"kc{ft}_{chk}")
            for t in range(CT // 2):
                nc.tensor.matmul(
                    ps, wq[:, 2 * t:2 * t + 2, C + ft * 128:C + (ft + 1) * 128],
                    hkvA[:, 2 * t:2 * t + 2, chk * 512:(chk + 1) * 512],
                    start=(t == 0), stop=(t == CT // 2 - 1),
                    perf_mode=mybir.MatmulPerfMode.DoubleRow)
                if t < CT // 2 - 1:
                    yield
            nc.vector.tensor_copy(
                out=kT[ft][:, chk * 512:(chk + 1) * 512], in_=ps)
            yield

        def q_chunk(ft, chk):
            ps = mps.tile([128, 512], F32, tag="mp", name=f"qc{ft}_{chk}")
            for t in range(CT // 2):
                nc.tensor.matmul(
                    ps, wq[:, 2 * t:2 * t + 2, ft * 128:(ft + 1) * 128],
                    hkvA[:, 2 * t:2 * t + 2, chk * 512:(chk + 1) * 512],
                    start=(t == 0), stop=(t == CT // 2 - 1),
                    perf_mode=mybir.MatmulPerfMode.DoubleRow)
                if t < CT // 2 - 1:
                    yield
            nc.vector.tensor_copy(
                out=qT[ft][:, chk * 512:(chk + 1) * 512], in_=ps)
            yield

        def run_gen(g):
            for _ in g:
                pass

        with (
            tc.tile_pool(name="xw", bufs=2) as xpool,
            tc.tile_pool(name="ln1_stat", bufs=6) as lstat,
        ):
            for g in range(KT // 4):  # groups of 4 token tiles (512 tokens)
                xts, xcs = [], []
                for j in range(4):
                    nt = 4 * g + j
                    xt = xpool.tile([128, C], F32, tag=f"xt{j}", name=f"xt{j}")
                    dma(out=xt, in_=xb[nt * 128:(nt + 1) * 128, :])
                    st = lstat.tile([128, 3, 6], F32, tag="st")
                    xg = xt.rearrange("p (s d) -> p s d", s=3)
                    for s in range(3):
                        nc.vector.bn_stats(out=st[:, s], in_=xg[:, s])
                    mv = lstat.tile([128, 2], F32, tag="mv")
                    nc.vector.bn_aggr(out=mv, in_=st)
                    rstd = lstat.tile([128, 1], F32, tag="rstd")
                    nc.scalar.activation(out=rstd, in_=mv[:, 1:2],
                                         func=AF.Sqrt, bias=eps_t, scale=1.0)
                    nc.vector.reciprocal(out=rstd, in_=rstd)
                    nmr = lstat.tile([128, 1], F32, tag="nmr")
                    nc.vector.tensor_scalar(out=nmr, in0=mv[:, 0:1],
                                            scalar1=-1.0, scalar2=rstd,
                                            op0=ALU.mult, op1=ALU.mult)
                    xc = xpool.tile([128, C], BF16, tag=f"xc{j}",
                                    name=f"xc{j}", bufs=1)
                    nc.scalar.activation(out=xc, in_=xt, func=AF.Identity,
                                         scale=rstd, bias=nmr)
                    xts.append(xt)
                    xcs.append(xc)
                # interleave raw-x^T (DVE drain) and h^T (ACT drain)
                # transposes so the two drain engines run concurrently.
                for ct in range(CT):
                    if g < NO // 512:
                        ps32 = mps.tile([128, 512], F32, tag="mp",
                                        name="ps32")
                        for j in range(4):
                            nc.tensor.transpose(
                                ps32[:, j * 128:(j + 1) * 128],
                                xts[j][:, ct * 128:(ct + 1) * 128], ident)
                        nc.vector.tensor_copy(
                            out=xoT[ct][:, g * 512:(g + 1) * 512], in_=ps32)
                    ps = mps.tile([128, 512], BF16, tag="mp", name="pst")
                    for j in range(4):
                        nc.tensor.transpose(
                            ps[:, j * 128:(j + 1) * 128],
                            xcs[j][:, ct * 128:(ct + 1) * 128], ident_bf)
                    nc.scalar.activation(
                        out=hkvT[ct][:, g * 512:(g + 1) * 512],
                        in_=ps, func=AF.Identity,
                        scale=g1_s[:, ct:ct + 1], bias=b1_s[:, ct:ct + 1])
                for ft in range(3):  # ft0-2 K here; ft3-5 are attn fillers
                    run_gen(k_chunk(ft, g))
                for j in range(4):
                    v_tile(4 * g + j)
                if g < QCH:
                    for ft in range(3):
                        run_gen(q_chunk(ft, g))

        # =================== attention machinery ===================
        def ot_norm_piece(ch, hp, rec, base):
            """Normalize one head pair's output into oT[ch][hp]."""
            t = oT_pool.tile([128, 512], BF16, tag="oT",
                             name=f"oT{ch}_{hp}")
            for i in range(2):
                r = 2 * hp + i
                rb = asm.tile([1, 512], BF16, tag="rb")
                dma(out=rb, in_=rec[r - base:r - base + 1, :])
                vb = asm.tile([D, 512], BF16, tag="vb")
                nc.gpsimd.partition_broadcast(vb, rb, channels=D)
                nc.vector.tensor_mul(t[qs[i], :], oU_all[ch][r], vb)
            oT[ch][hp] = t

        def rec_quarter(ch, q):
            """Batched reciprocal of 4 denominator rows (heads 4q..4q+3),
            then normalize head pairs 2q and 2q+1 -- runs as soon as each
            third of a chunk's denominators lands so oT is ready early."""
            rec = dent[ch][q]
            with nc.allow_low_precision(reason="softmax denom in bf16"):
                nc.vector.reciprocal(out=rec, in_=rec)
            for hp in (2 * q, 2 * q + 1):
                ot_norm_piece(ch, hp, rec, 4 * q)

        class Feed:
            """Deadline-ordered filler quanta. pull() emits one quantum;
            flush(hp) force-emits every generator due before head pair hp
            (PE is in-order, so a quantum consumed by hp's matmuls must be
            emitted before them)."""

            def __init__(self):
                self.items = []

            def add(self, deadline, gen):
                self.items.append([deadline, gen])

            def pull(self):
                for it in self.items:
                    if it[1] is not None:
                        try:
                            next(it[1])
                            return True
                        except StopIteration:
                            it[1] = None
                return False

            def flush(self, hp):
                for it in self.items:
                    if it[1] is not None and it[0] <= hp:
                        for _ in it[1]:
                            pass
                        it[1] = None

            def flush_all(self):
                self.flush(10 ** 9)

        def attention_all(feed_of, boundary_hook):
            """Both chunks as ONE continuously-skewed (ch, hp, nt) stream:
            the next head pair's S matmuls interleave with the previous
            pair's AV tail, so the exp stream never pauses at boundaries.
            feed_of(ch) -> Feed; boundary_hook(ch) fires before chunk ch's
            first S (pool swaps / weight prefetch / feed construction)."""
            seq = [(ch, hp, nt) for ch in range(QCH)
                   for hp in range(CT) for nt in range(KT)]
            po_cur = {}
            pts = {}
            feed = None
            for idx in range(len(seq) + SKEW):
                if idx < len(seq):
                    ch, hp, nt = seq[idx]
                    if nt == 0:
                        if hp == 0:
                            if feed is not None:
                                feed.flush_all()
                            boundary_hook(ch)
                            feed = feed_of(ch)
                            dent[ch] = [
                                den_pool.tile([4, 512], BF16, tag=f"dent{h}",
                                              name=f"dent{ch}_{h}")
                                for h in range(3)]
                        feed.flush(hp)
                    qch = slice(ch * 512, (ch + 1) * 512)
                    ps2 = sps.tile([128, 1024], F32, tag="ps")
                    for i in range(2):
                        nc.tensor.matmul(
                            ps2[:, i * 512:(i + 1) * 512],
                            kT[hp][qs[i], nt * 128:(nt + 1) * 128],
                            qT[hp][qs[i], qch],
                            start=True, stop=True,
                            tile_position=(64 * i, 0))
                    pt2 = p_sb.tile([128, 1024], BF16, tag="pt")
                    nc.scalar.activation(out=pt2, in_=ps2,
                                         func=AF.Exp, scale=ISCALE / (WQS * WQS))
                    pts[(ch, hp, nt)] = pt2
                if idx >= SKEW:
                    ch_a, hp_a, m = seq[idx - SKEW]
                    pt2 = pts.pop((ch_a, hp_a, m))
                    if m == 0:
                        po_cur[(ch_a, hp_a)] = [
                            ops.tile([D + 1, 512], F32, tag="po",
                                     name=f"po{ch_a}_{hp_a}_{i}")
                            for i in range(2)]
                    po = po_cur[(ch_a, hp_a)]
                    for i in range(2):
                        nc.tensor.matmul(
                            po[i], vA[m][:, 2 * hp_a + i, :],
                            pt2[:, i * 512:(i + 1) * 512],
                            start=(m == 0), stop=(m == KT - 1),
                            skip_group_check=True)
                    if m == KT - 1:
                        # drain: unnormalized O^T halves + denominator row
                        for i in range(2):
                            r = 2 * hp_a + i
                            oU = oU_pool.tile([D, 512], BF16, tag="oU",
                                              name=f"oU{ch_a}_{r}")
                            nc.vector.tensor_copy(out=oU, in_=po[i][0:D, :])
                            oU_all[ch_a][r] = oU
                            dt = asm.tile([1, 512], BF16, tag="dtmp")
                            nc.vector.tensor_copy(out=dt, in_=po[i][D:D + 1, :])
                            dma(out=dent[ch_a][r // 4]
                                [(r % 4):(r % 4) + 1, :], in_=dt)
                        del po_cur[(ch_a, hp_a)]
                        if hp_a % 2 == 1:
                            rec_quarter(ch_a, hp_a // 2)
                # filler pulls: ~1.5/step keeps PE full without starving
                # the exp stream, and drains inventory before chunk end
                feed.pull()
                if idx % 2 == 0:
                    feed.pull()

        # ------------- proj / LN2 / MLP building blocks -------------
        def proj_quanta(ch, wp):
            """proj + x2 residual for chunk ch. f0/f1 start on the first
            five oT tiles so quanta are ready before oT[5] lands."""
            cs = slice(ch * 512, (ch + 1) * 512)
            ps01 = []
            for f in range(2):
                ps = mps.tile([128, 512], F32, tag="mp", name=f"pj{f}")
                ps01.append(ps)
                for ct in range(CT - 1):
                    nc.tensor.matmul(
                        ps, wp[:, ct, f * 128:(f + 1) * 128], oT[ch][ct],
                        start=(ct == 0), stop=False)
                    yield
            for f in range(2):
                nc.tensor.matmul(
                    ps01[f], wp[:, CT - 1, f * 128:(f + 1) * 128],
                    oT[ch][CT - 1], start=False, stop=True)
                nc.vector.scalar_tensor_tensor(
                    out=x2T[f][:, cs], in0=ps01[f],
                    scalar=pb_s[:, f:f + 1], in1=xoT[f][:, cs],
                    op0=ALU.add, op1=ALU.add)
                yield
            for f in range(2, CT):
                ps = mps.tile([128, 512], F32, tag="mp", name=f"pj{f}")
                for ct in range(CT):
                    nc.tensor.matmul(
                        ps, wp[:, ct, f * 128:(f + 1) * 128], oT[ch][ct],
                        start=(ct == 0), stop=(ct == CT - 1))
                    if ct < CT - 1:
                        yield
                nc.vector.scalar_tensor_tensor(
                    out=x2T[f][:, cs], in0=ps,
                    scalar=pb_s[:, f:f + 1], in1=xoT[f][:, cs],
                    op0=ALU.add, op1=ALU.add)
                yield

        def ln2_quanta(ch, lw):
            cs = slice(ch * 512, (ch + 1) * 512)
            psum = mps.tile([1, 512], F32, tag="mp", name="psum")
            pssq = mps.tile([1, 512], F32, tag="mp", name="pssq")
            for ct in range(CT):
                sq = lw.tile([128, 512], BF16, tag="sq", bufs=1)
                nc.vector.tensor_mul(sq, x2T[ct][:, cs], x2T[ct][:, cs])
                nc.tensor.matmul(psum, ones_cb, x2T[ct][:, cs],
                                 start=(ct == 0), stop=(ct == CT - 1),
                                 skip_group_check=True)
                nc.tensor.matmul(pssq, ones_cb, sq,
                                 start=(ct == 0), stop=(ct == CT - 1),
                                 skip_group_check=True)
                yield
            # serial stats chain (sqrt lands at a head-pair boundary so the
            # one table round-trip hides under S/AV matmuls)
            mu = lw.tile([1, 512], F32, tag="mu", name=f"mu{ch}", bufs=1)
            nc.vector.tensor_scalar_mul(mu, psum, 1.0 / C)
            mu2 = lw.tile([1, 512], F32, tag="mu2", bufs=1)
            nc.vector.tensor_mul(mu2, mu, mu)
            var = lw.tile([1, 512], F32, tag="var", bufs=1)
            nc.vector.scalar_tensor_tensor(
                out=var, in0=pssq, scalar=1.0 / C, in1=mu2,
                op0=ALU.mult, op1=ALU.subtract)
            std = lw.tile([1, 512], F32, tag="mu2", name="std", bufs=1)
            nc.scalar.activation(out=std, in_=var, func=AF.Sqrt,
                                 bias=eps_t[0:1], scale=1.0)
            rstdf = lw.tile([1, 512], F32, tag="rstdf", bufs=1)
            nc.vector.reciprocal(out=rstdf, in_=std)
            nmr = lw.tile([1, 512], BF16, tag=f"nmr2_{ch}", name="nmr2",
                          bufs=1)
            nc.vector.scalar_tensor_tensor(
                out=nmr, in0=mu, scalar=-1.0, in1=rstdf,
                op0=ALU.mult, op1=ALU.mult)
            rstd = lw.tile([1, 512], BF16, tag=f"rstd2_{ch}", name="rstd2",
                           bufs=1)
            nc.vector.tensor_copy(out=rstd, in_=rstdf)
            ln2v[ch] = (rstd, nmr)
            yield

        def h2_quanta(ch, lw, h2_pool, bc_pool):
            """h2 = ((x2T*bc_r + bc_m)*g2 + b2): broadcast on gpsimd,
            affine on DVE (keeps ScalarE free for exps)."""
            cs = slice(ch * 512, (ch + 1) * 512)
            rstd, nmr = ln2v[ch]
            bc_r = bc_pool.tile([128, 512], BF16, tag="bc_r")
            nc.gpsimd.partition_broadcast(bc_r, rstd, channels=128)
            bc_m = bc_pool.tile([128, 512], BF16, tag="bc_m")
            nc.gpsimd.partition_broadcast(bc_m, nmr, channels=128)
            yield
            for ct in range(CT):
                t = lw.tile([128, 512], BF16, tag="h2tmp")
                nc.vector.tensor_mul(t, x2T[ct][:, cs], bc_r)
                nc.vector.tensor_add(t, t, bc_m)
                h2 = h2_pool.tile([128, 512], BF16,
                                  tag=f"h2_{ct}_{ch}", name=f"h2_{ct}_{ch}")
                nc.vector.tensor_scalar(
                    out=h2, in0=t, scalar1=g2_s[:, ct:ct + 1],
                    scalar2=b2_s[:, ct:ct + 1],
                    op0=ALU.mult, op1=ALU.add)
                h2c[ch][ct] = h2
                yield

        # fc1 matmuls only, staging pre-gelu h1 in bf16 (gelu deferred so
        # ScalarE stays on the exp table set during attention)
        def fc1_stage_quanta(ch, hb, w1t, gbuf, h1s):
            for ht in range(HQ):
                ps = mps.tile([128, 512], F32, tag="mp", name="f1")
                for ct in range(CT):
                    nc.tensor.matmul(
                        ps, w1t[:, ct, ht * 128:(ht + 1) * 128],
                        h2c[ch][ct],
                        start=(ct == 0), stop=(ct == CT - 1))
                    if ct < CT - 1:
                        yield
                h1 = gbuf.tile([128, 512], BF16, tag="gb",
                               name=f"h1_{ch}_{hb}_{ht}")
                nc.vector.tensor_copy(out=h1, in_=ps)
                h1s[ht] = h1
                yield

        def gelu_piece(ch, hb, gbuf, h1s, gs):
            for ht in range(HQ):
                g = gbuf.tile([128, 512], BF16, tag="gb",
                              name=f"g_{ch}_{hb}_{ht}")
                hti = hb * HQ + ht
                nc.scalar.activation(out=g, in_=h1s[ht], func=AF.Gelu,
                                     bias=f1b_s[:, hti:hti + 1], scale=1.0)
                gs[ht] = g

        # fc1 with gelu fused right off PSUM (post-attention phases)
        def fc1_full(ch, hb, w1t, gbuf, gs):
            for ht in range(HQ):
                ps = mps.tile([128, 512], F32, tag="mp", name="f1")
                for ct in range(CT):
                    nc.tensor.matmul(
                        ps, w1t[:, ct, ht * 128:(ht + 1) * 128],
                        h2c[ch][ct],
                        start=(ct == 0), stop=(ct == CT - 1))
                g = gbuf.tile([128, 512], BF16, tag="gb",
                              name=f"g_{ch}_{hb}_{ht}")
                hti = hb * HQ + ht
                nc.scalar.activation(out=g, in_=ps, func=AF.Gelu,
                                     bias=f1b_s[:, hti:hti + 1], scale=1.0)
                gs[ht] = g

        acc = {}

        def fc2_piece(ch, hb, w2t, gs, accp, osb):
            cs = slice(ch * 512, (ch + 1) * 512)
            for ft in range(CT):
                ps = mps.tile([128, 512], F32, tag="mp", name="f2")
                for ht in range(HQ):
                    nc.tensor.matmul(
                        ps, w2t[:, ht, ft * 128:(ft + 1) * 128], gs[ht],
                        start=(ht == 0), stop=(ht == HQ - 1))
                if hb == 0:
                    a = accp.tile([128, 512], BF16, tag=f"acc{ft}_{ch}",
                                  name=f"acc{ft}_{ch}")
                    acc[(ft, ch)] = a
                    nc.vector.scalar_tensor_tensor(
                        out=a, in0=ps,
                        scalar=f2b_s[:, ft:ft + 1],
                        in1=x2T[ft][:, cs],
                        op0=ALU.add, op1=ALU.add)
                else:
                    ot = osb.tile([128, 512], F32, tag="ot")
                    nc.vector.tensor_add(ot, ps, acc[(ft, ch)])
                    dma(out=outT[ft * 128:(ft + 1) * 128, cs], in_=ot)

        # =================== attention (both chunks, flat) ===================
        env = {}
        h1s0 = [None] * HQ  # staged pre-gelu fc1 outputs (ch0, hb0)
        HH = HID // 2

        def boundary(ch):
            if ch == 1:
                # hkvT/wq freed -> SBUF headroom for the MLP working set;
                # wproj + wfc1(h0) DMAs ride under chunk 1 (their quanta
                # start ~25us in).
                _hstk.close()
                env["wp_pool"] = pool(name="wproj", bufs=1, side="right")
                env["w1_pool"] = pool(name="wfc1", bufs=1, side="right")
                env["gbuf"] = pool(name="gbuf", bufs=13, side="right")
                env["h2a_pool"] = pool(name="h2a_sb", bufs=1, side="right")
                env["acc_pool"] = pool(name="acc_sb", bufs=1, side="right")
                env["bc_pool"] = pool(name="bc_sb", bufs=1, side="right")
                env["lw"] = pool(name="mlp_work", bufs=1, side="right")
                wp = env["wp_pool"].tile([128, CT, C], BF16, tag="wproj")
                dma(out=wp, in_=wprojT.rearrange("(ct p) f -> p ct f", p=128))
                env["wp"] = wp
                w1t0 = env["w1_pool"].tile([128, CT, HH], BF16, tag="wfc1",
                                           name="w1h0")
                dma(out=w1t0,
                    in_=wfc1T.rearrange("(ct p) f -> p ct f", p=128)
                    [:, :, 0:HH])
                env["w1t0"] = w1t0

        def mk_feed(ch):
            f = Feed()
            if ch == 0:
                for ft in (3, 4, 5):
                    for chk in range(4):
                        f.add(ft, k_chunk(ft, chk))
                    for chk in range(QCH):
                        f.add(ft, q_chunk(ft, chk))
            else:
                f.add(99, proj_quanta(0, env["wp"]))
                f.add(99, ln2_quanta(0, env["lw"]))
                f.add(99, h2_quanta(0, env["lw"], env["h2a_pool"],
                                    env["bc_pool"]))
                f.add(99, fc1_stage_quanta(0, 0, env["w1t0"], env["gbuf"],
                                           h1s0))
            return f

        attention_all(mk_feed, boundary)
        _stk.close()  # free qT/kT/vA/pt pools for the MLP tail
        wp, w1t0 = env["wp"], env["w1t0"]
        gbuf, lw = env["gbuf"], env["lw"]

        # =================== MLP tail ===================
        w2_pool = pool(name="wfc2", bufs=1, side="right")
        h2b_pool = pool(name="h2b_sb", bufs=1, side="right")
        accb_pool = pool(name="accb_sb", bufs=1, side="right")
        osb = pool(name="out_sb", bufs=2, side="right")
        w2t0 = w2_pool.tile([128, HQ, C], BF16, tag="wfc2", name="w2h0")
        dma(out=w2t0,
            in_=wfc2T.rearrange("(ht p) f -> p ht f", p=128)[:, 0:HQ, :])
        w1t1 = env["w1_pool"].tile([128, CT, HH], BF16, tag="wfc1b",
                                   name="w1h1")
        dma(out=w1t1,
            in_=wfc1T.rearrange("(ct p) f -> p ct f", p=128)[:, :, HH:HID])
        w2t1 = w2_pool.tile([128, HQ, C], BF16, tag="wfc2b", name="w2h1")
        dma(out=w2t1,
            in_=wfc2T.rearrange("(ht p) f -> p ht f", p=128)[:, HQ:HT, :])

        # gelu00 + fc2(0,0) first: fc2's f0 matmuls trail the gelu stream
        # (only tail work with no dependence on chunk1's oT), then proj(1)
        # once oT(1) lands, then the rest at full PE rate.
        g00 = [None] * HQ
        gelu_piece(0, 0, gbuf, h1s0, g00)
        fc2_piece(0, 0, w2t0, g00, env["acc_pool"], None)
        run_gen(proj_quanta(1, wp))
        run_gen(ln2_quanta(1, lw))
        g01 = [None] * HQ
        fc1_full(0, 1, w1t1, gbuf, g01)
        run_gen(h2_quanta(1, lw, h2b_pool, env["bc_pool"]))
        fc2_piece(0, 1, w2t1, g01, None, osb)
        g10 = [None] * HQ
        fc1_full(1, 0, w1t0, gbuf, g10)
        fc2_piece(1, 0, w2t0, g10, accb_pool, None)
        g11 = [None] * HQ
        fc1_full(1, 1, w1t1, gbuf, g11)
        fc2_piece(1, 1, w2t1, g11, None, osb)


def kernel(**inputs):
    global _NC_CACHE, LAST_RESULTS
    import os
    ins = {k: np.ascontiguousarray(np.asarray(v, dtype=np.float32))
           for k, v in inputs.items()}
    if _NC_CACHE is None:
        _NC_CACHE = build_program()
    nc = _NC_CACHE

    import ml_dtypes
    bf = ml_dtypes.bfloat16

    def col6(v):  # [768] -> [128, 6] matching "(ct p) -> p ct"
        return np.ascontiguousarray(v.reshape(6, 128).T)

    cvec = np.concatenate(
        [col6(ins["ln1_g"]), col6(ins["ln1_b"]), col6(ins["ln2_g"]),
         col6(ins["ln2_b"]), col6(ins["proj_b"]), col6(ins["fc2_b"]),
         np.ascontiguousarray(ins["fc1_b"].reshape(HT, 128).T)],
        axis=1).astype(np.float32)

    shared = {
        "wqkvT": np.ascontiguousarray(
            (ins["qkv_w"].T * 16.0).astype(ml_dtypes.float8_e4m3)),
        "wprojT": np.ascontiguousarray(ins["proj_w"].T.astype(bf)),
        "wfc1T": np.ascontiguousarray(ins["fc1_w"].T.astype(bf)),
        "wfc2T": np.ascontiguousarray(ins["fc2_w"].T.astype(bf)),
        "cvec": np.ascontiguousarray(cvec),
    }
    in_maps = []
    for s in range(NCORES):
        b, half = s // 2, s % 2
        m = dict(shared)
        m["xb"] = np.ascontiguousarray(np.roll(ins["x"][b], -half * NO, axis=0))
        in_maps.append(m)

    trace = bool(int(os.environ.get("KBENCH_TRACE", "0")))
    LAST_RESULTS = run_bass_kernel_spmd(
        nc, in_maps, core_ids=list(range(NCORES)), trace=trace)
    out = np.empty((B, N, C), np.float32)
    for s in range(NCORES):
        b, half = s // 2, s % 2
        out[b, half * NO:(half + 1) * NO, :] = LAST_RESULTS.results[s]["outT"].T
    return out
